# revision 19
# baseline (speedup 1.0000x reference)
"""Trainium2 Bass kernel for nn_AttentionBlock (GroupNorm + 2-head attention + proj + residual).

Full inputs: x (16, 256, 32, 32) f32, gn_w/gn_b (256,), wq/wk/wv/wp (256, 256).
Sharding: pure data-parallel over batch - 16 / 8 cores = 2 batch elements per core.
No collectives; outputs concatenated on host.

Per-core dataflow (per batch element, channels-on-partitions):
  xb (256, 1024) bf16 -> GroupNorm stats on DVE (reduce_sum + fused
  tensor_tensor_reduce for sum-of-squares), group combine via tiny PE matmuls,
  xn = xb*scale+bias on DVE (per-partition affine). q,k = Wq/Wk @ xn; vT tiles =
  xn_mt^T @ Wv. Attention per head: st_jt (j=128, i=1024) = k_jt^T q;
  et = exp(scale*st) on ACT; U (hd, i) and D (softmax denom, replicated)
  accumulate over jt in PSUM as (128, 512) half-tiles; ao = U * recip(D) on DVE.
  proj: out_psum = Wp_h0^T ao0 + Wp_h1^T ao1 + I^T xb (residual folded in as an
  identity matmul on the bf16 x), copied to SBUF and DMA'd out.

Scheduling (emission order = per-engine queue order):
  - input DMAs: xb tiles lead each queue (cbg/cbw behind them) so GN stats
    start ASAP; a dummy Sqrt preloads the ACT sqrt table before it's needed.
  - 9 cold warmup MMs trip the HAM clock gate, 6 bridge MMs abut the first QKV.
  - scores of one head weave instruction-by-instruction with U/D matmuls of the
    previous head / QKV of the next batch, so PE never waits on ACT's exp.
  - per-head U/D PSUM alternates between the 'ud' pool (head 0) and the 'qm'
    pool (head 1) so consecutive heads' U/D accumulations overlap.
PSUM budget (8 banks): st (128,1024)x2bufs = 4, u+d (128,512) = 2, qm x2 = 2.
"""

import numpy as np

import concourse.bass as bass
import concourse.tile as tile
from concourse import bacc, mybir
from concourse.bass_utils import run_bass_kernel_spmd

N_CORES = 8
B = 16
BPC = B // N_CORES  # batch elements per core
C = 256
H = W = 32
N = H * W  # 1024 spatial positions
HEADS = 2
HD = C // HEADS  # 128 head dim
G = 4  # groupnorm groups
GSIZE = C // G  # 64 channels per group
EPS = 1e-5
ATT_SCALE = float((C * HEADS) ** -0.5)
P = 128  # partitions
CT = C // P  # channel tiles (2)
FT = 512  # u/d half-tile free dim
JT = N // P  # j tiles (8)
NG = GSIZE * N  # elements per (batch, group)

# const blob column offsets; [0, CB_W) bf16 weight blob, [0, CB_G) fp32 GN blob.
OFF_W = 0  # 4 weights (q,k,v,p), each CT*C = 512 cols
OFF_ONES = 2048  # 128 cols of 1.0
OFF_ID = 2176  # 128x128 identity
CB_W = 2304
OFF_GNWB = 0  # per ct: 2 cols (gn_w, gn_b)
OFF_GMASK = 4  # per ct: G cols (group mask / NG)
OFF_GMT = 12  # per ct: 128 cols (mask^T, values in rows 0..G-1)
OFF_EPS = 268  # one col: EPS in rows 0..G-1
CB_G = 269

f32 = mybir.dt.float32
bf16 = mybir.dt.bfloat16
MM_DT = bf16
N_WARM1 = 9  # cold warmup MMs before the GN matmuls
N_WARM2 = 6  # bridge MMs between GN matmuls and first QKV matmul
AF = mybir.ActivationFunctionType
ALU = mybir.AluOpType
AX = mybir.AxisListType
USE_TTR = False  # tensor_tensor_reduce crashes TRN2 hw (NRT exec-unit error); use ACT Square


def build_bass(bpc=BPC):
    nc = bacc.Bacc("TRN2", target_bir_lowering=False, debug=False)

    xb_d = nc.dram_tensor("xb", [bpc, C, N], bf16, kind="ExternalInput").ap()
    cbw_d = nc.dram_tensor("cbw", [P, CB_W], MM_DT, kind="ExternalInput").ap()
    cbg_d = nc.dram_tensor("cbg", [P, CB_G], f32, kind="ExternalInput").ap()
    out_d = nc.dram_tensor("out", [bpc, C, N], f32, kind="ExternalOutput").ap()

    with tile.TileContext(nc) as tc:
        with (
            tc.tile_pool(name="consts", bufs=1) as consts,
            tc.tile_pool(name="xp", bufs=1) as xp,
            tc.tile_pool(name="xnp", bufs=1) as xnp,
            tc.tile_pool(name="qkp", bufs=1) as qkp,
            tc.tile_pool(name="vp", bufs=1) as vp,
            tc.tile_pool(name="etp", bufs=2) as etp,
            tc.tile_pool(name="aop", bufs=2) as aop,
            tc.tile_pool(name="smp", bufs=2) as smp,
            tc.tile_pool(name="pst", bufs=2, space="PSUM") as pst,
            tc.tile_pool(name="pud", bufs=1, space="PSUM") as pud,
            tc.tile_pool(name="pqm", bufs=2, space="PSUM") as pqm,
        ):
            # ---- SBUF constants + input DMAs.
            wt = consts.tile([P, FT], bf16, tag="warm")
            nc.gpsimd.memset(wt[:], 0.0)

            xbt = []
            for b in range(bpc):
                t = xp.tile([P, CT * N], bf16, tag=f"xb{b}", name=f"xb{b}")
                xbt.append(t)
            # xb tiles lead each queue; big/late consts behind them.
            nc.sync.dma_start(xbt[0][:, 0:N], xb_d[0, 0:P, :])
            cbw = consts.tile([P, CB_W], MM_DT, tag="cbw")
            nc.scalar.dma_start(xbt[0][:, N : 2 * N], xb_d[0, P : 2 * P, :])
            nc.scalar.dma_start(cbw[:], cbw_d[:])
            cbg = consts.tile([P, CB_G], f32, tag="cbg")
            nc.gpsimd.dma_start(cbg[:], cbg_d[:])
            if bpc > 1:
                nc.gpsimd.dma_start(xbt[1][:, 0:N], xb_d[1, 0:P, :])
                nc.gpsimd.dma_start(xbt[1][:, N : 2 * N], xb_d[1, P : 2 * P, :])

            def w_ap(i, kt):  # (128, C) lhsT slice of weight i, k-tile kt
                base = OFF_W + i * (CT * C) + kt * C
                return cbw[:, base : base + C]

            ones_ap = cbw[:, OFF_ONES : OFF_ONES + P]
            id_ap = cbw[:, OFF_ID : OFF_ID + P]
            gw = [cbg[:, OFF_GNWB + ct * 2 : OFF_GNWB + (ct + 1) * 2] for ct in range(CT)]
            gm = [cbg[:, OFF_GMASK + ct * G : OFF_GMASK + (ct + 1) * G] for ct in range(CT)]
            gmt = [cbg[0:G, OFF_GMT + ct * P : OFF_GMT + (ct + 1) * P] for ct in range(CT)]
            eps_ap = cbg[0:G, OFF_EPS : OFF_EPS + 1]
            WQ, WK, WV, WP_ = 0, 1, 2, 3

            # ---- warmup MMs (cold): trip the HAM clock gate.
            wps1 = pst.tile([P, FT], f32, tag="st")
            for _ in range(N_WARM1):
                nc.tensor.matmul(wps1[:], wt[:, 0:P], wt[:], start=True, stop=True)

            # ---- GroupNorm --------------------------------------------------
            s12_all = {}

            def gn_stats(b):
                s12s = []
                for ct in range(CT):
                    xsl = xbt[b][:, ct * N : (ct + 1) * N]
                    s12 = smp.tile([P, 2], f32, tag=f"s12_{ct}")
                    nc.vector.reduce_sum(s12[:, 0:1], xsl, AX.X)
                    if USE_TTR:
                        sq = smp.tile([P, N], bf16, tag="sq")
                        nc.vector.tensor_tensor_reduce(
                            out=sq[:], in0=xsl, in1=xsl, scale=1.0, scalar=0.0,
                            op0=ALU.mult, op1=ALU.add, accum_out=s12[:, 1:2],
                        )
                    else:
                        sq = smp.tile([P, N], f32, tag="sq")
                        nc.scalar.activation(sq[:], xsl, AF.Square, accum_out=s12[:, 1:2])
                    s12s.append(s12)
                s12_all[b] = s12s

            def gn_mm1(b):
                gstats = pqm.tile([G, 2], f32, tag="qm")
                for ct in range(CT):
                    nc.tensor.matmul(
                        gstats[:], gm[ct], s12_all[b][ct][:],
                        start=(ct == 0), stop=(ct == CT - 1),
                    )
                return gstats

            def gn_chain_pre(b, gstats):
                mrs = smp.tile([G, 2], f32, tag="mrs")  # col0 = rstd, col1 = mean
                nc.vector.tensor_copy(mrs[:, 1:2], gstats[:, 0:1])
                negvar = smp.tile([G, 1], f32, tag="negvar")
                nc.vector.scalar_tensor_tensor(
                    negvar[:], mrs[:, 1:2], mrs[:, 1:2], gstats[:, 1:2],
                    ALU.mult, ALU.subtract,
                )
                # rstd = exp(-0.5*ln(var+eps)): stays on the Exp ACT table set,
                # avoiding the Sqrt table load (and the DVE reciprocal hop).
                lnv = smp.tile([G, 1], f32, tag="std")
                nc.scalar.activation(lnv[:], negvar[:], AF.Ln, bias=eps_ap, scale=-1.0)
                nc.scalar.activation(mrs[:, 0:1], lnv[:], AF.Exp, scale=-0.5)
                return mrs

            def gn_post(b, mrs, xn_t):
                """bc matmuls + scale/bias + xn (DVE)."""
                for ct in range(CT):
                    bc = pqm.tile([P, 2], f32, tag="qm")
                    nc.tensor.matmul(bc[:], gmt[ct], mrs[:], start=True, stop=True)
                    scale = smp.tile([P, 1], f32, tag=f"scale{ct}")
                    nc.vector.tensor_tensor(scale[:], bc[:, 0:1], gw[ct][:, 0:1], ALU.mult)
                    nbias = smp.tile([P, 1], f32, tag=f"nbias{ct}")
                    nc.vector.tensor_tensor(nbias[:], bc[:, 1:2], scale[:], ALU.mult)
                    nc.vector.tensor_tensor(nbias[:], gw[ct][:, 1:2], nbias[:], ALU.subtract)
                    for nt in range(2):
                        sl = slice(ct * N + nt * FT, ct * N + (nt + 1) * FT)
                        nc.vector.tensor_scalar(
                            xn_t[:, sl], xbt[b][:, sl], scale[:], nbias[:],
                            ALU.mult, ALU.add,
                        )

            xn_all = {}

            # ---- QKV pieces -------------------------------------------------
            q_t, k_t, vT = {}, {}, {}

            def alloc_qk(b):
                q_t[b] = [qkp.tile([P, N], MM_DT, tag=f"q{b}{ot}", name=f"q{b}{ot}") for ot in range(CT)]
                k_t[b] = [qkp.tile([P, N], MM_DT, tag=f"k{b}{ot}", name=f"k{b}{ot}") for ot in range(CT)]

            def qk_chunk(b, wi, ot, cast_eng):
                """one (128,1024) psum + cast chunk for q or k, out tile ot."""
                dst = (q_t if wi == WQ else k_t)[b][ot]
                def c():
                    ps = pst.tile([P, N], f32, tag="st")
                    for nt in range(2):
                        sl = slice(nt * FT, (nt + 1) * FT)
                        for kt in range(CT):
                            nc.tensor.matmul(
                                ps[:, sl],
                                w_ap(wi, kt)[:, ot * P : (ot + 1) * P],
                                xn_all[b][:, kt * N + nt * FT : kt * N + (nt + 1) * FT],
                                start=(kt == 0), stop=(kt == CT - 1),
                            )
                    if cast_eng == "act":
                        nc.scalar.copy(dst[:], ps[:])
                    else:
                        nc.vector.tensor_copy(dst[:], ps[:])
                return c

            def v_chunks(b):
                vT[b] = vp.tile([P, JT * C], MM_DT, tag=f"vt{b}", name=f"vt{b}")
                chunks = []
                for mt0 in range(0, JT, 2):
                    def c(mt0=mt0, b=b):
                        for mt in (mt0, mt0 + 1):
                            ps = pqm.tile([P, C], f32, tag="qm")
                            for kt in range(CT):
                                nc.tensor.matmul(
                                    ps[:],
                                    xn_all[b][:, kt * N + mt * P : kt * N + (mt + 1) * P],
                                    w_ap(WV, kt),
                                    start=(kt == 0), stop=(kt == CT - 1),
                                )
                            nc.vector.tensor_copy(vT[b][:, mt * C : (mt + 1) * C], ps[:])
                    chunks.append(c)
                return chunks

            # ---- attention pieces ------------------------------------------
            def sc_items(b, h, et_tile, act_extras=None):
                """8 items; act_extras[jt] = list of ACT closures to emit after
                exp jt (used to slot sqrt/k-casts into the exp stream)."""
                items = []
                for jt in range(JT):
                    def s(jt=jt, b=b, h=h, et_tile=et_tile):
                        st = pst.tile([P, N], f32, tag="st")
                        for nt in range(2):
                            sl = slice(nt * FT, (nt + 1) * FT)
                            nc.tensor.matmul(
                                st[:, sl],
                                k_t[b][h][:, jt * P : (jt + 1) * P],
                                q_t[b][h][:, sl],
                                start=True, stop=True,
                            )
                        nc.scalar.activation(
                            et_tile[:, jt * N : (jt + 1) * N], st[:],
                            AF.Exp, scale=ATT_SCALE,
                        )
                        if act_extras and jt in act_extras:
                            for fn in act_extras[jt]:
                                fn()
                    items.append(s)
                return items

            ao_t = {}

            def du_chunks(b, h, et_tile, pool_tag):
                """per half: 8 jt chunks (d MM + u MM) + 1 ao chunk (DVE).
                pool_tag selects ('ud' pool) or ('qm' pool) for U/D psum."""
                if (b, h) not in ao_t:
                    ao_t[(b, h)] = aop.tile([P, N], MM_DT, tag=f"ao{h}", name=f"ao{b}{h}")
                ao = ao_t[(b, h)]
                ud = {}
                chunks = []
                for half in range(2):
                    for jt in range(JT):
                        def c(half=half, jt=jt, b=b, h=h, et_tile=et_tile):
                            if jt == 0:
                                if pool_tag == "ud":
                                    ud[half] = (
                                        pud.tile([P, FT], f32, tag="u", name="u"),
                                        pud.tile([P, FT], f32, tag="d", name="d"),
                                    )
                                else:
                                    ud[half] = (
                                        pqm.tile([P, FT], f32, tag="qm", name="uq"),
                                        pqm.tile([P, FT], f32, tag="qm", name="dq"),
                                    )
                            u_, d_ = ud[half]
                            sl = et_tile[:, jt * N + half * FT : jt * N + half * FT + FT]
                            nc.tensor.matmul(
                                d_[:], ones_ap, sl, start=(jt == 0), stop=(jt == JT - 1)
                            )
                            nc.tensor.matmul(
                                u_[:],
                                vT[b][:, jt * C + h * HD : jt * C + h * HD + HD],
                                sl,
                                start=(jt == 0), stop=(jt == JT - 1),
                            )
                        chunks.append(c)
                    def ao_c(half=half):
                        u_, d_ = ud[half]
                        r = smp.tile([P, FT], f32, tag="r")
                        nc.vector.reciprocal_approx_fast(out=r[:], in_=d_[:])
                        nc.vector.tensor_tensor(
                            ao[:, half * FT : (half + 1) * FT], u_[:], r[:], ALU.mult
                        )
                    chunks.append(ao_c)
                return chunks

            def proj_chunks(b, pool="qm", split=0):
                """split>0: for the first `split` (ot,nt) tiles, emit the
                h0+identity matmuls as early chunks and defer the h1 matmul +
                copy + DMA, so the PE can fill the last softmax-normalize wait."""
                engs = [nc.sync, nc.gpsimd] if b == 0 else [nc.sync, nc.scalar]
                tags = ["qm", "qm"] if pool == "qm" else ["u", "d"]
                heads_t, tails_t = [], []
                for idx, (ot, nt) in enumerate([(o, n) for o in range(CT) for n in range(2)]):
                    state = {}
                    def c_head(ot=ot, nt=nt, b=b, idx=idx, state=state):
                        if pool == "qm":
                            pj = pqm.tile([P, FT], f32, tag="qm", name="pj")
                        else:
                            pj = pud.tile([P, FT], f32, tag=tags[idx % 2], name="pj")
                        state["pj"] = pj
                        nc.tensor.matmul(
                            pj[:],
                            w_ap(WP_, 0)[:, ot * P : (ot + 1) * P],
                            ao_t[(b, 0)][:, nt * FT : (nt + 1) * FT],
                            start=True, stop=False,
                        )
                        nc.tensor.matmul(
                            pj[:], id_ap,
                            xbt[b][:, ot * N + nt * FT : ot * N + (nt + 1) * FT],
                            start=False, stop=False,
                        )
                    def c_tail(ot=ot, nt=nt, b=b, idx=idx, state=state):
                        pj = state["pj"]
                        sl = slice(nt * FT, (nt + 1) * FT)
                        nc.tensor.matmul(
                            pj[:],
                            w_ap(WP_, 1)[:, ot * P : (ot + 1) * P],
                            ao_t[(b, 1)][:, sl],
                            start=False, stop=True,
                        )
                        ot_sb = smp.tile([P, FT], f32, tag="osb", name="osb")
                        nc.vector.tensor_copy(ot_sb[:], pj[:])
                        engs[idx % 2].dma_start(
                            out_d[b, ot * P : (ot + 1) * P, sl], ot_sb[:]
                        )
                    if idx < split:
                        heads_t.append(c_head)
                        tails_t.append(c_tail)
                    else:
                        def c_full(ch=c_head, ct_=c_tail):
                            ch()
                            ct_()
                        tails_t.append(c_full)
                return heads_t + tails_t

            def weave(score_it, fillers):
                fi = 0
                ns = max(1, len(score_it))
                for i, s in enumerate(score_it):
                    s()
                    target = (i + 1) * len(fillers) // ns
                    while fi < target:
                        fillers[fi]()
                        fi += 1
                while fi < len(fillers):
                    fillers[fi]()
                    fi += 1

            # ---- global schedule -------------------------------------------
            # GN b0 (DVE stats overlap warmup MMs)
            gn_stats(0)
            gst0 = gn_mm1(0)
            mrs0 = gn_chain_pre(0, gst0)
            xn_all[0] = xnp.tile([P, CT * N], bf16, tag="xn0", name="xn0")
            gn_post(0, mrs0, xn_all[0])

            wps2 = pst.tile([P, FT], f32, tag="st")
            for _ in range(N_WARM2):
                nc.tensor.matmul(wps2[:], wt[:, 0:P], wt[:], start=True, stop=True)

            # q/k of b0 head 0 first (unblocks scores b0h0), casts: q on DVE, k on ACT
            alloc_qk(0)
            qk_chunk(0, WQ, 0, "dve")()
            qk_chunk(0, WK, 0, "act")()

            # b1 GN stats on DVE behind b0's xn
            def gn1_stats_dve():
                if bpc > 1:
                    gn_stats(1)

            gn1_mm_done = {}

            def gn1_mm_chunk():
                if bpc > 1:
                    gst1 = gn_mm1(1)
                    mrs1 = gn_chain_pre(1, gst1)
                    xn_all[1] = xnp.tile([P, CT * N], bf16, tag="xn1", name="xn1")
                    gn_post(1, mrs1, xn_all[1])

            gn1_stats_dve()

            # P2: scores b0h0 woven with v0, q/k ot1, gn1 matmuls, qkv b1
            et = {}
            et[(0, 0)] = etp.tile([P, JT * N], MM_DT, tag="et", name="et00")
            fill = []
            v0 = v_chunks(0)
            fill += v0[:2]
            fill.append(qk_chunk(0, WQ, 1, "dve"))
            fill.append(qk_chunk(0, WK, 1, "dve"))
            fill += v0[2:]
            fill.append(gn1_mm_chunk)
            if bpc > 1:
                alloc_qk(1)
                fill.append(qk_chunk(1, WQ, 0, "dve"))
                fill.append(qk_chunk(1, WK, 0, "dve"))
                fill.append(qk_chunk(1, WQ, 1, "dve"))
                fill.append(qk_chunk(1, WK, 1, "dve"))
                fill += v_chunks(1)
            weave(sc_items(0, 0, et[(0, 0)]), fill)

            # P3: scores b0h1 woven with du b0h0 [ud pool]
            et[(0, 1)] = etp.tile([P, JT * N], MM_DT, tag="et", name="et01")
            weave(sc_items(0, 1, et[(0, 1)]), du_chunks(0, 0, et[(0, 0)], "ud"))
            if bpc > 1:
                # P4: scores b1h0 woven with du b0h1 [qm pool]
                et[(1, 0)] = etp.tile([P, JT * N], MM_DT, tag="et", name="et10")
                weave(sc_items(1, 0, et[(1, 0)]), du_chunks(0, 1, et[(0, 1)], "qm"))
                # P5: scores b1h1 woven with proj b0 + du b1h0 [ud] + du b1h1 half0 [qm]
                et[(1, 1)] = etp.tile([P, JT * N], MM_DT, tag="et", name="et11")
                du11 = du_chunks(1, 1, et[(1, 1)], "qm")
                weave(
                    sc_items(1, 1, et[(1, 1)]),
                    proj_chunks(0) + du_chunks(1, 0, et[(1, 0)], "ud") + du11[:6],
                )
                # P6: drain du b1h1 [qm]; proj b1 on the freed ud banks, with the
                # first two tiles' h0+identity matmuls filling the ao(1,1) wait.
                p1 = proj_chunks(1, pool="ud", split=2)
                for c in du11[6:]:
                    c()
                for c in p1:
                    c()
            else:
                for c in du_chunks(0, 1, et[(0, 1)], "qm"):
                    c()
                for c in proj_chunks(0):
                    c()

    nc.compile()
    return nc


def build_const_blob(gn_w, gn_b, wq, wk, wv, wp):
    """Returns (cbw bf16 [P, CB_W], cbg f32 [P, CB_G])."""
    import ml_dtypes

    cbw = np.zeros((P, CB_W), np.float32)
    for i, wmat in enumerate((wq, wk, wv, wp)):
        wT = np.asarray(wmat, np.float32).T  # (c_in, c_out)
        for kt in range(CT):
            cbw[:, OFF_W + i * CT * C + kt * C : OFF_W + i * CT * C + (kt + 1) * C] = (
                wT[kt * P : (kt + 1) * P, :]
            )
    cbw[:, OFF_ONES : OFF_ONES + P] = 1.0
    cbw[:, OFF_ID : OFF_ID + P] = np.eye(P, dtype=np.float32)
    cbg = np.zeros((P, CB_G), np.float32)
    cbg[:, OFF_GNWB + 0 : OFF_GNWB + 4 : 2] = np.asarray(gn_w, np.float32).reshape(CT, P).T
    cbg[:, OFF_GNWB + 1 : OFF_GNWB + 4 : 2] = np.asarray(gn_b, np.float32).reshape(CT, P).T
    for ct in range(CT):
        for p in range(P):
            g = (ct * P + p) // GSIZE
            cbg[p, OFF_GMASK + ct * G + g] = 1.0 / NG
            cbg[g, OFF_GMT + ct * P + p] = 1.0
    cbg[0:G, OFF_EPS] = EPS
    return cbw.astype(ml_dtypes.bfloat16), cbg


_NC_CACHE = {}


def kernel(x, gn_w, gn_b, wq, wk, wv, wp):
    import ml_dtypes

    x = np.ascontiguousarray(np.asarray(x, dtype=np.float32))
    b, c, h, w = x.shape
    xrb = x.reshape(b, c, h * w).astype(ml_dtypes.bfloat16)
    cbw, cbg = build_const_blob(gn_w, gn_b, wq, wk, wv, wp)

    if "nc" not in _NC_CACHE:
        _NC_CACHE["nc"] = build_bass()
    nc = _NC_CACHE["nc"]

    in_maps = [
        dict(
            xb=np.ascontiguousarray(xrb[i * BPC : (i + 1) * BPC]),
            cbw=cbw,
            cbg=cbg,
        )
        for i in range(N_CORES)
    ]
    res = run_bass_kernel_spmd(nc, in_maps, list(range(N_CORES)))
    out = np.concatenate([res.results[i]["out"] for i in range(N_CORES)], axis=0)
    return out.reshape(b, c, h, w).astype(np.float32)


if __name__ == "__main__":
    rng = np.random.default_rng(0)
    ins = {
        "x": rng.standard_normal((B, C, H, W), dtype=np.float32),
        "gn_w": np.ones((C,), np.float32),
        "gn_b": np.zeros((C,), np.float32),
        "wq": rng.standard_normal((C, C), dtype=np.float32) * C**-0.5,
        "wk": rng.standard_normal((C, C), dtype=np.float32) * C**-0.5,
        "wv": rng.standard_normal((C, C), dtype=np.float32) * C**-0.5,
        "wp": rng.standard_normal((C, C), dtype=np.float32) * C**-0.5,
    }
    out = kernel(**ins)
    print(out.shape, out.dtype)


# revision 21
# speedup vs baseline: 1.2352x; 1.2352x over previous
"""Trainium2 Bass kernel for nn_AttentionBlock (GroupNorm + 2-head attention + proj + residual).

Full inputs: x (16, 256, 32, 32) f32, gn_w/gn_b (256,), wq/wk/wv/wp (256, 256).
Sharding: pure data-parallel over batch - 16 / 8 cores = 2 batch elements per core.
No collectives; outputs concatenated on host.

Per-core dataflow (per batch element, channels-on-partitions):
  xb (256, 1024) bf16 -> GroupNorm stats on DVE (reduce_sum + fused
  tensor_tensor_reduce for sum-of-squares), group combine via tiny PE matmuls,
  xn = xb*scale+bias on DVE (per-partition affine). q,k = Wq/Wk @ xn; vT tiles =
  xn_mt^T @ Wv. Attention per head: st_jt (j=128, i=1024) = k_jt^T q;
  et = exp(scale*st) on ACT; U (hd, i) and D (softmax denom, replicated)
  accumulate over jt in PSUM as (128, 512) half-tiles; ao = U * recip(D) on DVE.
  proj: out_psum = Wp_h0^T ao0 + Wp_h1^T ao1 + I^T xb (residual folded in as an
  identity matmul on the bf16 x), copied to SBUF and DMA'd out.

Scheduling (emission order = per-engine queue order):
  - input DMAs: xb tiles lead each queue (cbg/cbw behind them) so GN stats
    start ASAP; a dummy Sqrt preloads the ACT sqrt table before it's needed.
  - 9 cold warmup MMs trip the HAM clock gate, 6 bridge MMs abut the first QKV.
  - scores of one head weave instruction-by-instruction with U/D matmuls of the
    previous head / QKV of the next batch, so PE never waits on ACT's exp.
  - per-head U/D PSUM alternates between the 'ud' pool (head 0) and the 'qm'
    pool (head 1) so consecutive heads' U/D accumulations overlap.
PSUM budget (8 banks): st (128,1024)x2bufs = 4, u+d (128,512) = 2, qm x2 = 2.
"""

import numpy as np

import concourse.bass as bass
import concourse.tile as tile
from concourse import bacc, mybir
from concourse.bass_utils import run_bass_kernel_spmd

N_CORES = 8
B = 16
BPC = B // N_CORES  # batch elements per core
C = 256
H = W = 32
N = H * W  # 1024 spatial positions
HEADS = 2
HD = C // HEADS  # 128 head dim
G = 4  # groupnorm groups
GSIZE = C // G  # 64 channels per group
EPS = 1e-5
ATT_SCALE = float((C * HEADS) ** -0.5)
P = 128  # partitions
CT = C // P  # channel tiles (2)
FT = 512  # u/d half-tile free dim
JT = N // P  # j tiles (8)
NG = GSIZE * N  # elements per (batch, group)

# const blob column offsets; [0, CB_W) bf16 weight blob, [0, CB_G) fp32 GN blob.
OFF_W = 0  # 4 weights (q,k,v,p), each CT*C = 512 cols
OFF_ONES = 2048  # 128 cols of 1.0
OFF_ID = 2176  # 128x128 identity
CB_W = 2304
OFF_GNWB = 0  # per ct: 2 cols (gn_w, gn_b)
OFF_GMASK = 4  # per ct: G cols (group mask / NG)
OFF_GMT = 12  # per ct: 128 cols (mask^T, values in rows 0..G-1)
OFF_EPS = 268  # one col: EPS in rows 0..G-1
CB_G = 269

f32 = mybir.dt.float32
bf16 = mybir.dt.bfloat16
MM_DT = bf16
N_WARM1 = 9  # cold warmup MMs before the GN matmuls
N_WARM2 = 6  # bridge MMs between GN matmuls and first QKV matmul
AF = mybir.ActivationFunctionType
ALU = mybir.AluOpType
AX = mybir.AxisListType
USE_TTR = False  # tensor_tensor_reduce crashes TRN2 hw (NRT exec-unit error); use ACT Square


def build_bass(bpc=BPC):
    nc = bacc.Bacc("TRN2", target_bir_lowering=False, debug=False)

    xb_d = nc.dram_tensor("xb", [bpc, C, N], bf16, kind="ExternalInput").ap()
    cbw_d = nc.dram_tensor("cbw", [P, CB_W], MM_DT, kind="ExternalInput").ap()
    cbg_d = nc.dram_tensor("cbg", [P, CB_G], f32, kind="ExternalInput").ap()
    out_d = nc.dram_tensor("out", [bpc, C, N], f32, kind="ExternalOutput").ap()

    with tile.TileContext(nc) as tc:
        with (
            tc.tile_pool(name="consts", bufs=1) as consts,
            tc.tile_pool(name="xp", bufs=1) as xp,
            tc.tile_pool(name="xnp", bufs=1) as xnp,
            tc.tile_pool(name="qkp", bufs=1) as qkp,
            tc.tile_pool(name="vp", bufs=1) as vp,
            tc.tile_pool(name="etp", bufs=2) as etp,
            tc.tile_pool(name="aop", bufs=2) as aop,
            tc.tile_pool(name="smp", bufs=2) as smp,
            tc.tile_pool(name="pst", bufs=2, space="PSUM") as pst,
            tc.tile_pool(name="pud", bufs=1, space="PSUM") as pud,
            tc.tile_pool(name="pqm", bufs=2, space="PSUM") as pqm,
        ):
            # ---- SBUF constants + input DMAs.
            wt = consts.tile([P, FT], bf16, tag="warm")
            nc.gpsimd.memset(wt[:], 0.0)

            xbt = []
            for b in range(bpc):
                t = xp.tile([P, CT * N], bf16, tag=f"xb{b}", name=f"xb{b}")
                xbt.append(t)
            # xb tiles lead each queue; big/late consts behind them.
            nc.sync.dma_start(xbt[0][:, 0:N], xb_d[0, 0:P, :])
            cbw = consts.tile([P, CB_W], MM_DT, tag="cbw")
            nc.scalar.dma_start(xbt[0][:, N : 2 * N], xb_d[0, P : 2 * P, :])
            nc.scalar.dma_start(cbw[:], cbw_d[:])
            cbg = consts.tile([P, CB_G], f32, tag="cbg")
            nc.gpsimd.dma_start(cbg[:], cbg_d[:])
            if bpc > 1:
                nc.gpsimd.dma_start(xbt[1][:, 0:N], xb_d[1, 0:P, :])
                nc.scalar.dma_start(xbt[1][:, N : 2 * N], xb_d[1, P : 2 * P, :])

            # dummy Sqrt: preload the sqrt ACT table before the GN chain needs it
            dum = smp.tile([G, 1], f32, tag="dum")
            nc.scalar.activation(dum[:], wt[0:G, 0:1], AF.Sqrt)

            def w_ap(i, kt):  # (128, C) lhsT slice of weight i, k-tile kt
                base = OFF_W + i * (CT * C) + kt * C
                return cbw[:, base : base + C]

            ones_ap = cbw[:, OFF_ONES : OFF_ONES + P]
            id_ap = cbw[:, OFF_ID : OFF_ID + P]
            gw = [cbg[:, OFF_GNWB + ct * 2 : OFF_GNWB + (ct + 1) * 2] for ct in range(CT)]
            gm = [cbg[:, OFF_GMASK + ct * G : OFF_GMASK + (ct + 1) * G] for ct in range(CT)]
            gmt = [cbg[0:G, OFF_GMT + ct * P : OFF_GMT + (ct + 1) * P] for ct in range(CT)]
            eps_ap = cbg[0:G, OFF_EPS : OFF_EPS + 1]
            WQ, WK, WV, WP_ = 0, 1, 2, 3

            # ---- warmup MMs (cold): trip the HAM clock gate.
            wps1 = pst.tile([P, FT], f32, tag="st")
            for _ in range(N_WARM1):
                nc.tensor.matmul(wps1[:], wt[:, 0:P], wt[:], start=True, stop=True)

            # ---- GroupNorm --------------------------------------------------
            s12_all = {}

            def gn_stats(b):
                s12s = []
                for ct in range(CT):
                    xsl = xbt[b][:, ct * N : (ct + 1) * N]
                    s12 = smp.tile([P, 2], f32, tag=f"s12_{ct}")
                    nc.vector.reduce_sum(s12[:, 0:1], xsl, AX.X)
                    if USE_TTR:
                        sq = smp.tile([P, N], bf16, tag="sq")
                        nc.vector.tensor_tensor_reduce(
                            out=sq[:], in0=xsl, in1=xsl, scale=1.0, scalar=0.0,
                            op0=ALU.mult, op1=ALU.add, accum_out=s12[:, 1:2],
                        )
                    else:
                        sq = smp.tile([P, N], f32, tag="sq")
                        nc.scalar.activation(sq[:], xsl, AF.Square, accum_out=s12[:, 1:2])
                    s12s.append(s12)
                s12_all[b] = s12s

            def gn_mm1(b):
                gstats = pqm.tile([G, 2], f32, tag="qm")
                for ct in range(CT):
                    nc.tensor.matmul(
                        gstats[:], gm[ct], s12_all[b][ct][:],
                        start=(ct == 0), stop=(ct == CT - 1),
                    )
                return gstats

            def gn_chain_pre(b, gstats):
                mrs = smp.tile([G, 2], f32, tag="mrs")  # col0 = rstd, col1 = mean
                nc.vector.tensor_copy(mrs[:, 1:2], gstats[:, 0:1])
                negvar = smp.tile([G, 1], f32, tag="negvar")
                nc.vector.scalar_tensor_tensor(
                    negvar[:], mrs[:, 1:2], mrs[:, 1:2], gstats[:, 1:2],
                    ALU.mult, ALU.subtract,
                )
                std = smp.tile([G, 1], f32, tag="std")
                nc.scalar.activation(std[:], negvar[:], AF.Sqrt, bias=eps_ap, scale=-1.0)
                nc.vector.reciprocal(mrs[:, 0:1], std[:])
                return mrs

            def gn_post(b, mrs, xn_t):
                """bc matmuls + scale/bias + xn (DVE)."""
                for ct in range(CT):
                    bc = pqm.tile([P, 2], f32, tag="qm")
                    nc.tensor.matmul(bc[:], gmt[ct], mrs[:], start=True, stop=True)
                    scale = smp.tile([P, 1], f32, tag=f"scale{ct}")
                    nc.vector.tensor_tensor(scale[:], bc[:, 0:1], gw[ct][:, 0:1], ALU.mult)
                    nbias = smp.tile([P, 1], f32, tag=f"nbias{ct}")
                    nc.vector.tensor_tensor(nbias[:], bc[:, 1:2], scale[:], ALU.mult)
                    nc.vector.tensor_tensor(nbias[:], gw[ct][:, 1:2], nbias[:], ALU.subtract)
                    for nt in range(2):
                        sl = slice(ct * N + nt * FT, ct * N + (nt + 1) * FT)
                        nc.vector.tensor_scalar(
                            xn_t[:, sl], xbt[b][:, sl], scale[:], nbias[:],
                            ALU.mult, ALU.add,
                        )

            xn_all = {}

            # ---- QKV pieces -------------------------------------------------
            q_t, k_t, vT = {}, {}, {}

            def alloc_qk(b):
                q_t[b] = [qkp.tile([P, N], MM_DT, tag=f"q{b}{ot}", name=f"q{b}{ot}") for ot in range(CT)]
                k_t[b] = [qkp.tile([P, N], MM_DT, tag=f"k{b}{ot}", name=f"k{b}{ot}") for ot in range(CT)]

            def qk_chunk(b, wi, ot, cast_eng):
                """one (128,1024) psum + cast chunk for q or k, out tile ot."""
                dst = (q_t if wi == WQ else k_t)[b][ot]
                def c():
                    ps = pst.tile([P, N], f32, tag="st")
                    for nt in range(2):
                        sl = slice(nt * FT, (nt + 1) * FT)
                        for kt in range(CT):
                            nc.tensor.matmul(
                                ps[:, sl],
                                w_ap(wi, kt)[:, ot * P : (ot + 1) * P],
                                xn_all[b][:, kt * N + nt * FT : kt * N + (nt + 1) * FT],
                                start=(kt == 0), stop=(kt == CT - 1),
                            )
                    if cast_eng == "act":
                        nc.scalar.copy(dst[:], ps[:])
                    else:
                        nc.vector.tensor_copy(dst[:], ps[:])
                return c

            def v_chunks(b):
                vT[b] = vp.tile([P, JT * C], MM_DT, tag=f"vt{b}", name=f"vt{b}")
                chunks = []
                for mt0 in range(0, JT, 2):
                    def c(mt0=mt0, b=b):
                        for mt in (mt0, mt0 + 1):
                            ps = pqm.tile([P, C], f32, tag="qm")
                            for kt in range(CT):
                                nc.tensor.matmul(
                                    ps[:],
                                    xn_all[b][:, kt * N + mt * P : kt * N + (mt + 1) * P],
                                    w_ap(WV, kt),
                                    start=(kt == 0), stop=(kt == CT - 1),
                                )
                            nc.vector.tensor_copy(vT[b][:, mt * C : (mt + 1) * C], ps[:])
                    chunks.append(c)
                return chunks

            # ---- attention pieces ------------------------------------------
            def sc_items(b, h, et_tile, act_extras=None):
                """8 items; act_extras[jt] = list of ACT closures to emit after
                exp jt (used to slot sqrt/k-casts into the exp stream)."""
                items = []
                for jt in range(JT):
                    def s(jt=jt, b=b, h=h, et_tile=et_tile):
                        st = pst.tile([P, N], f32, tag="st")
                        for nt in range(2):
                            sl = slice(nt * FT, (nt + 1) * FT)
                            nc.tensor.matmul(
                                st[:, sl],
                                k_t[b][h][:, jt * P : (jt + 1) * P],
                                q_t[b][h][:, sl],
                                start=True, stop=True,
                            )
                        nc.scalar.activation(
                            et_tile[:, jt * N : (jt + 1) * N], st[:],
                            AF.Exp, scale=ATT_SCALE,
                        )
                        if act_extras and jt in act_extras:
                            for fn in act_extras[jt]:
                                fn()
                    items.append(s)
                return items

            ao_t = {}

            def du_chunks(b, h, et_tile, pool_tag):
                """per half: 8 jt chunks (d MM + u MM) + 1 ao chunk (DVE).
                pool_tag selects ('ud' pool) or ('qm' pool) for U/D psum."""
                if (b, h) not in ao_t:
                    ao_t[(b, h)] = aop.tile([P, N], MM_DT, tag=f"ao{h}", name=f"ao{b}{h}")
                ao = ao_t[(b, h)]
                ud = {}
                chunks = []
                for half in range(2):
                    for jt in range(JT):
                        def c(half=half, jt=jt, b=b, h=h, et_tile=et_tile):
                            if jt == 0:
                                if pool_tag == "ud":
                                    ud[half] = (
                                        pud.tile([P, FT], f32, tag="u", name="u"),
                                        pud.tile([P, FT], f32, tag="d", name="d"),
                                    )
                                else:
                                    ud[half] = (
                                        pqm.tile([P, FT], f32, tag="qm", name="uq"),
                                        pqm.tile([P, FT], f32, tag="qm", name="dq"),
                                    )
                            u_, d_ = ud[half]
                            sl = et_tile[:, jt * N + half * FT : jt * N + half * FT + FT]
                            nc.tensor.matmul(
                                d_[:], ones_ap, sl, start=(jt == 0), stop=(jt == JT - 1)
                            )
                            nc.tensor.matmul(
                                u_[:],
                                vT[b][:, jt * C + h * HD : jt * C + h * HD + HD],
                                sl,
                                start=(jt == 0), stop=(jt == JT - 1),
                            )
                        chunks.append(c)
                    def ao_c(half=half):
                        u_, d_ = ud[half]
                        r = smp.tile([P, FT], f32, tag="r")
                        nc.vector.reciprocal_approx_fast(out=r[:], in_=d_[:])
                        nc.vector.tensor_tensor(
                            ao[:, half * FT : (half + 1) * FT], u_[:], r[:], ALU.mult
                        )
                    chunks.append(ao_c)
                return chunks

            def proj_chunks(b, pool="qm", split=0):
                """split>0: for the first `split` (ot,nt) tiles, emit the
                h0+identity matmuls as early chunks and defer the h1 matmul +
                copy + DMA, so the PE can fill the last softmax-normalize wait."""
                engs = [nc.sync, nc.gpsimd] if b == 0 else [nc.sync, nc.scalar]
                tags = ["qm", "qm"] if pool == "qm" else ["u", "d"]
                heads_t, tails_t = [], []
                for idx, (ot, nt) in enumerate([(o, n) for o in range(CT) for n in range(2)]):
                    state = {}
                    def c_head(ot=ot, nt=nt, b=b, idx=idx, state=state):
                        if pool == "qm":
                            pj = pqm.tile([P, FT], f32, tag="qm", name="pj")
                        else:
                            pj = pud.tile([P, FT], f32, tag=tags[idx % 2], name="pj")
                        state["pj"] = pj
                        nc.tensor.matmul(
                            pj[:],
                            w_ap(WP_, 0)[:, ot * P : (ot + 1) * P],
                            ao_t[(b, 0)][:, nt * FT : (nt + 1) * FT],
                            start=True, stop=False,
                        )
                        nc.tensor.matmul(
                            pj[:], id_ap,
                            xbt[b][:, ot * N + nt * FT : ot * N + (nt + 1) * FT],
                            start=False, stop=False,
                        )
                    def c_tail(ot=ot, nt=nt, b=b, idx=idx, state=state):
                        pj = state["pj"]
                        sl = slice(nt * FT, (nt + 1) * FT)
                        nc.tensor.matmul(
                            pj[:],
                            w_ap(WP_, 1)[:, ot * P : (ot + 1) * P],
                            ao_t[(b, 1)][:, sl],
                            start=False, stop=True,
                        )
                        ot_sb = smp.tile([P, FT], f32, tag="osb", name="osb")
                        nc.vector.tensor_copy(ot_sb[:], pj[:])
                        engs[idx % 2].dma_start(
                            out_d[b, ot * P : (ot + 1) * P, sl], ot_sb[:]
                        )
                    if idx < split:
                        heads_t.append(c_head)
                        tails_t.append(c_tail)
                    else:
                        def c_full(ch=c_head, ct_=c_tail):
                            ch()
                            ct_()
                        tails_t.append(c_full)
                return heads_t + tails_t

            def weave(score_it, fillers):
                fi = 0
                ns = max(1, len(score_it))
                for i, s in enumerate(score_it):
                    s()
                    target = (i + 1) * len(fillers) // ns
                    while fi < target:
                        fillers[fi]()
                        fi += 1
                while fi < len(fillers):
                    fillers[fi]()
                    fi += 1

            # ---- global schedule -------------------------------------------
            # GN b0 (DVE stats overlap warmup MMs)
            gn_stats(0)
            gst0 = gn_mm1(0)
            mrs0 = gn_chain_pre(0, gst0)
            xn_all[0] = xnp.tile([P, CT * N], bf16, tag="xn0", name="xn0")
            gn_post(0, mrs0, xn_all[0])

            wps2 = pst.tile([P, FT], f32, tag="st")
            for _ in range(N_WARM2):
                nc.tensor.matmul(wps2[:], wt[:, 0:P], wt[:], start=True, stop=True)

            # q/k of b0 head 0 first (unblocks scores b0h0), casts: q on DVE, k on ACT
            alloc_qk(0)
            qk_chunk(0, WQ, 0, "dve")()
            qk_chunk(0, WK, 0, "act")()

            # b1 GN stats on DVE behind b0's xn
            def gn1_stats_dve():
                if bpc > 1:
                    gn_stats(1)

            gn1_mm_done = {}

            def gn1_mm_chunk():
                if bpc > 1:
                    gst1 = gn_mm1(1)
                    mrs1 = gn_chain_pre(1, gst1)
                    xn_all[1] = xnp.tile([P, CT * N], bf16, tag="xn1", name="xn1")
                    gn_post(1, mrs1, xn_all[1])

            gn1_stats_dve()

            # P2: scores b0h0 woven with v0, q/k ot1, gn1 matmuls, qkv b1
            et = {}
            et[(0, 0)] = etp.tile([P, JT * N], MM_DT, tag="et", name="et00")
            fill = []
            v0 = v_chunks(0)
            fill += v0[:2]
            fill.append(qk_chunk(0, WQ, 1, "dve"))
            fill.append(qk_chunk(0, WK, 1, "dve"))
            fill += v0[2:]
            fill.append(gn1_mm_chunk)
            if bpc > 1:
                alloc_qk(1)
                fill.append(qk_chunk(1, WQ, 0, "dve"))
                fill.append(qk_chunk(1, WK, 0, "dve"))
                fill.append(qk_chunk(1, WQ, 1, "dve"))
                fill.append(qk_chunk(1, WK, 1, "dve"))
                fill += v_chunks(1)
            weave(sc_items(0, 0, et[(0, 0)]), fill)

            # P3: scores b0h1 woven with du b0h0 [ud pool]
            et[(0, 1)] = etp.tile([P, JT * N], MM_DT, tag="et", name="et01")
            weave(sc_items(0, 1, et[(0, 1)]), du_chunks(0, 0, et[(0, 0)], "ud"))
            if bpc > 1:
                # P4: scores b1h0 woven with du b0h1 [qm pool]
                et[(1, 0)] = etp.tile([P, JT * N], MM_DT, tag="et", name="et10")
                weave(sc_items(1, 0, et[(1, 0)]), du_chunks(0, 1, et[(0, 1)], "qm"))
                # P5: scores b1h1 woven with proj b0 + du b1h0 [ud] + du b1h1 half0 [qm]
                et[(1, 1)] = etp.tile([P, JT * N], MM_DT, tag="et", name="et11")
                du11 = du_chunks(1, 1, et[(1, 1)], "qm")
                weave(
                    sc_items(1, 1, et[(1, 1)]),
                    proj_chunks(0) + du_chunks(1, 0, et[(1, 0)], "ud") + du11[:6],
                )
                # P6: drain du b1h1 [qm]; proj b1 on the freed ud banks, with the
                # first two tiles' h0+identity matmuls filling the ao(1,1) wait.
                p1 = proj_chunks(1, pool="ud", split=2)
                for c in du11[6:]:
                    c()
                for c in p1:
                    c()
            else:
                for c in du_chunks(0, 1, et[(0, 1)], "qm"):
                    c()
                for c in proj_chunks(0):
                    c()

    nc.compile()
    return nc


def build_const_blob(gn_w, gn_b, wq, wk, wv, wp):
    """Returns (cbw bf16 [P, CB_W], cbg f32 [P, CB_G])."""
    import ml_dtypes

    cbw = np.zeros((P, CB_W), np.float32)
    for i, wmat in enumerate((wq, wk, wv, wp)):
        wT = np.asarray(wmat, np.float32).T  # (c_in, c_out)
        for kt in range(CT):
            cbw[:, OFF_W + i * CT * C + kt * C : OFF_W + i * CT * C + (kt + 1) * C] = (
                wT[kt * P : (kt + 1) * P, :]
            )
    cbw[:, OFF_ONES : OFF_ONES + P] = 1.0
    cbw[:, OFF_ID : OFF_ID + P] = np.eye(P, dtype=np.float32)
    cbg = np.zeros((P, CB_G), np.float32)
    cbg[:, OFF_GNWB + 0 : OFF_GNWB + 4 : 2] = np.asarray(gn_w, np.float32).reshape(CT, P).T
    cbg[:, OFF_GNWB + 1 : OFF_GNWB + 4 : 2] = np.asarray(gn_b, np.float32).reshape(CT, P).T
    for ct in range(CT):
        for p in range(P):
            g = (ct * P + p) // GSIZE
            cbg[p, OFF_GMASK + ct * G + g] = 1.0 / NG
            cbg[g, OFF_GMT + ct * P + p] = 1.0
    cbg[0:G, OFF_EPS] = EPS
    return cbw.astype(ml_dtypes.bfloat16), cbg


_NC_CACHE = {}


def kernel(x, gn_w, gn_b, wq, wk, wv, wp):
    import ml_dtypes

    x = np.ascontiguousarray(np.asarray(x, dtype=np.float32))
    b, c, h, w = x.shape
    xrb = x.reshape(b, c, h * w).astype(ml_dtypes.bfloat16)
    cbw, cbg = build_const_blob(gn_w, gn_b, wq, wk, wv, wp)

    if "nc" not in _NC_CACHE:
        _NC_CACHE["nc"] = build_bass()
    nc = _NC_CACHE["nc"]

    in_maps = [
        dict(
            xb=np.ascontiguousarray(xrb[i * BPC : (i + 1) * BPC]),
            cbw=cbw,
            cbg=cbg,
        )
        for i in range(N_CORES)
    ]
    res = run_bass_kernel_spmd(nc, in_maps, list(range(N_CORES)))
    out = np.concatenate([res.results[i]["out"] for i in range(N_CORES)], axis=0)
    return out.reshape(b, c, h, w).astype(np.float32)


if __name__ == "__main__":
    rng = np.random.default_rng(0)
    ins = {
        "x": rng.standard_normal((B, C, H, W), dtype=np.float32),
        "gn_w": np.ones((C,), np.float32),
        "gn_b": np.zeros((C,), np.float32),
        "wq": rng.standard_normal((C, C), dtype=np.float32) * C**-0.5,
        "wk": rng.standard_normal((C, C), dtype=np.float32) * C**-0.5,
        "wv": rng.standard_normal((C, C), dtype=np.float32) * C**-0.5,
        "wp": rng.standard_normal((C, C), dtype=np.float32) * C**-0.5,
    }
    out = kernel(**ins)
    print(out.shape, out.dtype)


# revision 27
# speedup vs baseline: 1.2402x; 1.0040x over previous
"""Trainium2 Bass kernel for nn_AttentionBlock (GroupNorm + 2-head attention + proj + residual).

Full inputs: x (16, 256, 32, 32) f32, gn_w/gn_b (256,), wq/wk/wv/wp (256, 256).
Sharding: pure data-parallel over batch - 16 / 8 cores = 2 batch elements per core.
No collectives; outputs concatenated on host.

Per-core dataflow (per batch element, channels-on-partitions):
  xb (256, 1024) bf16 -> GroupNorm stats on DVE (reduce_sum + fused
  tensor_tensor_reduce for sum-of-squares), group combine via tiny PE matmuls,
  xn = xb*scale+bias on DVE (per-partition affine). q,k = Wq/Wk @ xn; vT tiles =
  xn_mt^T @ Wv. Attention per head: st_jt (j=128, i=1024) = k_jt^T q;
  et = exp(scale*st) on ACT; U (hd, i) and D (softmax denom, replicated)
  accumulate over jt in PSUM as (128, 512) half-tiles; ao = U * recip(D) on DVE.
  proj: out_psum = Wp_h0^T ao0 + Wp_h1^T ao1 + I^T xb (residual folded in as an
  identity matmul on the bf16 x), copied to SBUF and DMA'd out.

Scheduling (emission order = per-engine queue order):
  - input DMAs: xb tiles lead each queue (cbg/cbw behind them) so GN stats
    start ASAP; a dummy Sqrt preloads the ACT sqrt table before it's needed.
  - 9 cold warmup MMs trip the HAM clock gate, 6 bridge MMs abut the first QKV.
  - scores of one head weave instruction-by-instruction with U/D matmuls of the
    previous head / QKV of the next batch, so PE never waits on ACT's exp.
  - per-head U/D PSUM alternates between the 'ud' pool (head 0) and the 'qm'
    pool (head 1) so consecutive heads' U/D accumulations overlap.
PSUM budget (8 banks): st (128,1024)x2bufs = 4, u+d (128,512) = 2, qm x2 = 2.
"""

import numpy as np

import concourse.bass as bass
import concourse.tile as tile
from concourse import bacc, mybir
from concourse.bass_utils import run_bass_kernel_spmd

N_CORES = 8
B = 16
BPC = B // N_CORES  # batch elements per core
C = 256
H = W = 32
N = H * W  # 1024 spatial positions
HEADS = 2
HD = C // HEADS  # 128 head dim
G = 4  # groupnorm groups
GSIZE = C // G  # 64 channels per group
EPS = 1e-5
ATT_SCALE = float((C * HEADS) ** -0.5)
P = 128  # partitions
CT = C // P  # channel tiles (2)
FT = 512  # u/d half-tile free dim
JT = N // P  # j tiles (8)
NG = GSIZE * N  # elements per (batch, group)

# const blob column offsets; [0, CB_W) bf16 weight blob, [0, CB_G) fp32 GN blob.
OFF_W = 0  # 4 weights (q,k,v,p), each CT*C = 512 cols
OFF_ONES = 2048  # 128 cols of 1.0
OFF_ID = 2176  # 128x128 identity
CB_W = 2304
OFF_GNWB = 0  # per ct: 2 cols (gn_w, gn_b)
OFF_GMASK = 4  # per ct: G cols (group mask / NG)
OFF_GMT = 12  # per ct: 128 cols (mask^T, values in rows 0..G-1)
OFF_EPS = 268  # one col: EPS in rows 0..G-1
CB_G = 269

f32 = mybir.dt.float32
bf16 = mybir.dt.bfloat16
MM_DT = bf16
N_WARM1 = 9  # cold warmup MMs before the GN matmuls
N_WARM2 = 6  # bridge MMs between GN matmuls and first QKV matmul
AF = mybir.ActivationFunctionType
ALU = mybir.AluOpType
AX = mybir.AxisListType
USE_TTR = False  # tensor_tensor_reduce crashes TRN2 hw (NRT exec-unit error); use ACT Square


def build_bass(bpc=BPC):
    nc = bacc.Bacc("TRN2", target_bir_lowering=False, debug=False)

    xb_d = nc.dram_tensor("xb", [bpc, C, N], bf16, kind="ExternalInput").ap()
    cbw_d = nc.dram_tensor("cbw", [P, CB_W], MM_DT, kind="ExternalInput").ap()
    cbg_d = nc.dram_tensor("cbg", [P, CB_G], f32, kind="ExternalInput").ap()
    out_d = nc.dram_tensor("out", [bpc, C, N], f32, kind="ExternalOutput").ap()

    with tile.TileContext(nc) as tc:
        with (
            tc.tile_pool(name="consts", bufs=1) as consts,
            tc.tile_pool(name="xp", bufs=1) as xp,
            tc.tile_pool(name="xnp", bufs=1) as xnp,
            tc.tile_pool(name="qkp", bufs=1) as qkp,
            tc.tile_pool(name="vp", bufs=1) as vp,
            tc.tile_pool(name="etp", bufs=2) as etp,
            tc.tile_pool(name="aop", bufs=2) as aop,
            tc.tile_pool(name="smp", bufs=2) as smp,
            tc.tile_pool(name="pst", bufs=2, space="PSUM") as pst,
            tc.tile_pool(name="pud", bufs=1, space="PSUM") as pud,
            tc.tile_pool(name="pqm", bufs=2, space="PSUM") as pqm,
        ):
            # ---- SBUF constants + input DMAs.
            wt = consts.tile([P, FT], bf16, tag="warm")
            nc.gpsimd.memset(wt[:], 0.0)

            xbt = []
            for b in range(bpc):
                t = xp.tile([P, CT * N], bf16, tag=f"xb{b}", name=f"xb{b}")
                xbt.append(t)
            # xb tiles lead each queue; big/late consts behind them.
            nc.sync.dma_start(xbt[0][:, 0:N], xb_d[0, 0:P, :])
            cbw = consts.tile([P, CB_W], MM_DT, tag="cbw")
            nc.scalar.dma_start(xbt[0][:, N : 2 * N], xb_d[0, P : 2 * P, :])
            nc.scalar.dma_start(cbw[:], cbw_d[:])
            cbg = consts.tile([P, CB_G], f32, tag="cbg")
            nc.sync.dma_start(cbg[:], cbg_d[:])
            if bpc > 1:
                nc.scalar.dma_start(xbt[1][:, 0:N], xb_d[1, 0:P, :])
                nc.scalar.dma_start(xbt[1][:, N : 2 * N], xb_d[1, P : 2 * P, :])

            # dummy Sqrt: preload the sqrt ACT table before the GN chain needs it
            dum = smp.tile([G, 1], f32, tag="dum")
            nc.scalar.activation(dum[:], wt[0:G, 0:1], AF.Sqrt)

            def w_ap(i, kt):  # (128, C) lhsT slice of weight i, k-tile kt
                base = OFF_W + i * (CT * C) + kt * C
                return cbw[:, base : base + C]

            ones_ap = cbw[:, OFF_ONES : OFF_ONES + P]
            id_ap = cbw[:, OFF_ID : OFF_ID + P]
            gw = [cbg[:, OFF_GNWB + ct * 2 : OFF_GNWB + (ct + 1) * 2] for ct in range(CT)]
            gm = [cbg[:, OFF_GMASK + ct * G : OFF_GMASK + (ct + 1) * G] for ct in range(CT)]
            gmt = [cbg[0:G, OFF_GMT + ct * P : OFF_GMT + (ct + 1) * P] for ct in range(CT)]
            eps_ap = cbg[0:G, OFF_EPS : OFF_EPS + 1]
            WQ, WK, WV, WP_ = 0, 1, 2, 3

            # ---- warmup MMs (cold): trip the HAM clock gate.
            wps1 = pst.tile([P, FT], f32, tag="st")
            for _ in range(N_WARM1):
                nc.tensor.matmul(wps1[:], wt[:, 0:P], wt[:], start=True, stop=True)

            # ---- GroupNorm --------------------------------------------------
            s12_all = {}

            def gn_stats(b, s1_eng="dve"):
                """s1_eng='act' keeps batch-1 stats entirely off the DVE so the
                compiler cannot hoist them ahead of batch-0's GN chain ops."""
                s12s = []
                for ct in range(CT):
                    xsl = xbt[b][:, ct * N : (ct + 1) * N]
                    s12 = smp.tile([P, 2], f32, tag=f"s12_{ct}")
                    sq = smp.tile([P, N], f32, tag="sq")
                    if s1_eng == "act":
                        cp = smp.tile([P, N], bf16, tag="cp")
                        nc.scalar.activation(cp[:], xsl, AF.Copy, accum_out=s12[:, 0:1])
                    else:
                        nc.vector.reduce_sum(s12[:, 0:1], xsl, AX.X)
                    nc.scalar.activation(sq[:], xsl, AF.Square, accum_out=s12[:, 1:2])
                    s12s.append(s12)
                s12_all[b] = s12s

            def gn_mm1(b):
                gstats = pqm.tile([G, 2], f32, tag="qm")
                for ct in range(CT):
                    nc.tensor.matmul(
                        gstats[:], gm[ct], s12_all[b][ct][:],
                        start=(ct == 0), stop=(ct == CT - 1),
                    )
                return gstats

            def gn_chain_pre(b, gstats, defer_recip=False):
                mrs = smp.tile([G, 2], f32, tag="mrs")  # col0 = rstd, col1 = mean
                nc.vector.tensor_copy(mrs[:, 1:2], gstats[:, 0:1])
                negvar = smp.tile([G, 1], f32, tag="negvar")
                nc.vector.scalar_tensor_tensor(
                    negvar[:], mrs[:, 1:2], mrs[:, 1:2], gstats[:, 1:2],
                    ALU.mult, ALU.subtract,
                )
                std = smp.tile([G, 1], f32, tag="std")
                nc.scalar.activation(std[:], negvar[:], AF.Sqrt, bias=eps_ap, scale=-1.0)
                if defer_recip:
                    return (mrs, std)
                nc.vector.reciprocal(mrs[:, 0:1], std[:])
                return mrs

            def gn_post(b, mrs, xn_t):
                """bc matmuls + scale/bias + xn (DVE)."""
                for ct in range(CT):
                    bc = pqm.tile([P, 2], f32, tag="qm")
                    nc.tensor.matmul(bc[:], gmt[ct], mrs[:], start=True, stop=True)
                    scale = smp.tile([P, 1], f32, tag=f"scale{ct}")
                    nc.vector.tensor_tensor(scale[:], bc[:, 0:1], gw[ct][:, 0:1], ALU.mult)
                    nbias = smp.tile([P, 1], f32, tag=f"nbias{ct}")
                    nc.vector.tensor_tensor(nbias[:], bc[:, 1:2], scale[:], ALU.mult)
                    nc.vector.tensor_tensor(nbias[:], gw[ct][:, 1:2], nbias[:], ALU.subtract)
                    for nt in range(2):
                        sl = slice(ct * N + nt * FT, ct * N + (nt + 1) * FT)
                        nc.vector.tensor_scalar(
                            xn_t[:, sl], xbt[b][:, sl], scale[:], nbias[:],
                            ALU.mult, ALU.add,
                        )

            xn_all = {}

            # ---- QKV pieces -------------------------------------------------
            q_t, k_t, vT = {}, {}, {}

            def alloc_qk(b):
                q_t[b] = [qkp.tile([P, N], MM_DT, tag=f"q{b}{ot}", name=f"q{b}{ot}") for ot in range(CT)]
                k_t[b] = [qkp.tile([P, N], MM_DT, tag=f"k{b}{ot}", name=f"k{b}{ot}") for ot in range(CT)]

            def qk_chunk(b, wi, ot, cast_eng):
                """one (128,1024) psum + cast chunk for q or k, out tile ot."""
                dst = (q_t if wi == WQ else k_t)[b][ot]
                def c():
                    ps = pst.tile([P, N], f32, tag="st")
                    for nt in range(2):
                        sl = slice(nt * FT, (nt + 1) * FT)
                        for kt in range(CT):
                            nc.tensor.matmul(
                                ps[:, sl],
                                w_ap(wi, kt)[:, ot * P : (ot + 1) * P],
                                xn_all[b][:, kt * N + nt * FT : kt * N + (nt + 1) * FT],
                                start=(kt == 0), stop=(kt == CT - 1),
                            )
                    if cast_eng == "act":
                        nc.scalar.copy(dst[:], ps[:])
                    else:
                        nc.vector.tensor_copy(dst[:], ps[:])
                return c

            def v_chunks(b):
                vT[b] = vp.tile([P, JT * C], MM_DT, tag=f"vt{b}", name=f"vt{b}")
                chunks = []
                for mt0 in range(0, JT, 2):
                    def c(mt0=mt0, b=b):
                        for mt in (mt0, mt0 + 1):
                            ps = pqm.tile([P, C], f32, tag="qm")
                            for kt in range(CT):
                                nc.tensor.matmul(
                                    ps[:],
                                    xn_all[b][:, kt * N + mt * P : kt * N + (mt + 1) * P],
                                    w_ap(WV, kt),
                                    start=(kt == 0), stop=(kt == CT - 1),
                                )
                            nc.vector.tensor_copy(vT[b][:, mt * C : (mt + 1) * C], ps[:])
                    chunks.append(c)
                return chunks

            # ---- attention pieces ------------------------------------------
            def sc_items(b, h, et_tile, act_extras=None):
                """8 items; act_extras[jt] = list of ACT closures to emit after
                exp jt (used to slot sqrt/k-casts into the exp stream)."""
                items = []
                for jt in range(JT):
                    def s(jt=jt, b=b, h=h, et_tile=et_tile):
                        st = pst.tile([P, N], f32, tag="st")
                        for nt in range(2):
                            sl = slice(nt * FT, (nt + 1) * FT)
                            nc.tensor.matmul(
                                st[:, sl],
                                k_t[b][h][:, jt * P : (jt + 1) * P],
                                q_t[b][h][:, sl],
                                start=True, stop=True,
                            )
                        nc.scalar.activation(
                            et_tile[:, jt * N : (jt + 1) * N], st[:],
                            AF.Exp, scale=ATT_SCALE,
                        )
                        if act_extras and jt in act_extras:
                            for fn in act_extras[jt]:
                                fn()
                    items.append(s)
                return items

            ao_t = {}

            def du_chunks(b, h, et_tile, pool_tag):
                """per half: 8 jt chunks (d MM + u MM) + 1 ao chunk (DVE).
                pool_tag selects ('ud' pool) or ('qm' pool) for U/D psum."""
                if (b, h) not in ao_t:
                    ao_t[(b, h)] = aop.tile([P, N], MM_DT, tag=f"ao{h}", name=f"ao{b}{h}")
                ao = ao_t[(b, h)]
                ud = {}
                chunks = []
                for half in range(2):
                    for jt in range(JT):
                        def c(half=half, jt=jt, b=b, h=h, et_tile=et_tile):
                            if jt == 0:
                                if pool_tag == "ud":
                                    ud[half] = (
                                        pud.tile([P, FT], f32, tag="u", name="u"),
                                        pud.tile([P, FT], f32, tag="d", name="d"),
                                    )
                                else:
                                    ud[half] = (
                                        pqm.tile([P, FT], f32, tag="qm", name="uq"),
                                        pqm.tile([P, FT], f32, tag="qm", name="dq"),
                                    )
                            u_, d_ = ud[half]
                            sl = et_tile[:, jt * N + half * FT : jt * N + half * FT + FT]
                            nc.tensor.matmul(
                                d_[:], ones_ap, sl, start=(jt == 0), stop=(jt == JT - 1)
                            )
                            nc.tensor.matmul(
                                u_[:],
                                vT[b][:, jt * C + h * HD : jt * C + h * HD + HD],
                                sl,
                                start=(jt == 0), stop=(jt == JT - 1),
                            )
                        chunks.append(c)
                    def ao_c(half=half):
                        u_, d_ = ud[half]
                        r = smp.tile([P, FT], f32, tag="r")
                        nc.vector.reciprocal_approx_fast(out=r[:], in_=d_[:])
                        nc.vector.tensor_tensor(
                            ao[:, half * FT : (half + 1) * FT], u_[:], r[:], ALU.mult
                        )
                    chunks.append(ao_c)
                return chunks

            def proj_chunks(b, pool="qm", split=0):
                """split>0: for the first `split` (ot,nt) tiles, emit the
                h0+identity matmuls as early chunks and defer the h1 matmul +
                copy + DMA, so the PE can fill the last softmax-normalize wait."""
                engs = [nc.sync, nc.gpsimd] if b == 0 else [nc.sync, nc.scalar]
                tags = ["qm", "qm"] if pool == "qm" else ["u", "d"]
                heads_t, tails_t = [], []
                for idx, (ot, nt) in enumerate([(o, n) for o in range(CT) for n in range(2)]):
                    state = {}
                    def c_head(ot=ot, nt=nt, b=b, idx=idx, state=state):
                        if pool == "qm":
                            pj = pqm.tile([P, FT], f32, tag="qm", name="pj")
                        else:
                            pj = pud.tile([P, FT], f32, tag=tags[idx % 2], name="pj")
                        state["pj"] = pj
                        nc.tensor.matmul(
                            pj[:],
                            w_ap(WP_, 0)[:, ot * P : (ot + 1) * P],
                            ao_t[(b, 0)][:, nt * FT : (nt + 1) * FT],
                            start=True, stop=False,
                        )
                        nc.tensor.matmul(
                            pj[:], id_ap,
                            xbt[b][:, ot * N + nt * FT : ot * N + (nt + 1) * FT],
                            start=False, stop=False,
                        )
                    def c_tail(ot=ot, nt=nt, b=b, idx=idx, state=state):
                        pj = state["pj"]
                        sl = slice(nt * FT, (nt + 1) * FT)
                        nc.tensor.matmul(
                            pj[:],
                            w_ap(WP_, 1)[:, ot * P : (ot + 1) * P],
                            ao_t[(b, 1)][:, sl],
                            start=False, stop=True,
                        )
                        ot_sb = smp.tile([P, FT], f32, tag="osb", name="osb")
                        nc.vector.tensor_copy(ot_sb[:], pj[:])
                        engs[idx % 2].dma_start(
                            out_d[b, ot * P : (ot + 1) * P, sl], ot_sb[:]
                        )
                    if idx < split:
                        heads_t.append(c_head)
                        tails_t.append(c_tail)
                    else:
                        def c_full(ch=c_head, ct_=c_tail):
                            ch()
                            ct_()
                        tails_t.append(c_full)
                return heads_t + tails_t

            def weave(score_it, fillers):
                fi = 0
                ns = max(1, len(score_it))
                for i, s in enumerate(score_it):
                    s()
                    target = (i + 1) * len(fillers) // ns
                    while fi < target:
                        fillers[fi]()
                        fi += 1
                while fi < len(fillers):
                    fillers[fi]()
                    fi += 1

            # ---- global schedule -------------------------------------------
            # GN b0 (DVE stats overlap warmup MMs)
            gn_stats(0)
            gst0 = gn_mm1(0)
            mrs0 = gn_chain_pre(0, gst0)
            xn_all[0] = xnp.tile([P, CT * N], bf16, tag="xn0", name="xn0")
            gn_post(0, mrs0, xn_all[0])

            wps2 = pst.tile([P, FT], f32, tag="st")
            for _ in range(N_WARM2):
                nc.tensor.matmul(wps2[:], wt[:, 0:P], wt[:], start=True, stop=True)

            # q/k of b0 head 0 first (unblocks scores b0h0), casts on DVE
            alloc_qk(0)
            qk_chunk(0, WQ, 0, "dve")()
            qk_chunk(0, WK, 0, "dve")()

            # b1 GN stats on ACT (nothing on DVE to hoist); v0 matmuls keep PE busy
            v0 = v_chunks(0)
            if bpc > 1:
                gn_stats(1, s1_eng="act")
            for c in v0[:2]:
                c()

            mrs1_box = {}
            if bpc > 1:
                # gstats1 matmul + sqrt1 BEFORE the first exp: keeps the ACT
                # sqrt/square table resident until all sqrt work is done, so the
                # exp table set loads exactly once.
                gst1 = gn_mm1(1)
                mrs1_box["mrs"], mrs1_box["std"] = gn_chain_pre(1, gst1, defer_recip=True)

            def gn1_post_chunk():
                if bpc > 1:
                    mrs1 = mrs1_box["mrs"]
                    nc.vector.reciprocal(mrs1[:, 0:1], mrs1_box["std"][:])
                    xn_all[1] = xnp.tile([P, CT * N], bf16, tag="xn1", name="xn1")
                    gn_post(1, mrs1, xn_all[1])

            # P2: scores b0h0 woven with v0 rest, q/k ot1, gn1 post, qkv b1
            et = {}
            et[(0, 0)] = etp.tile([P, JT * N], MM_DT, tag="et", name="et00")
            fill = []
            fill += v0[2:]
            fill.append(qk_chunk(0, WQ, 1, "dve"))
            fill.append(qk_chunk(0, WK, 1, "dve"))
            fill.append(gn1_post_chunk)
            if bpc > 1:
                alloc_qk(1)
                fill.append(qk_chunk(1, WQ, 0, "dve"))
                fill.append(qk_chunk(1, WK, 0, "dve"))
                fill.append(qk_chunk(1, WQ, 1, "dve"))
                fill.append(qk_chunk(1, WK, 1, "dve"))
                fill += v_chunks(1)
            weave(sc_items(0, 0, et[(0, 0)]), fill)

            # P3: scores b0h1 woven with du b0h0 [ud pool]
            et[(0, 1)] = etp.tile([P, JT * N], MM_DT, tag="et", name="et01")
            weave(sc_items(0, 1, et[(0, 1)]), du_chunks(0, 0, et[(0, 0)], "ud"))
            if bpc > 1:
                # P4: scores b1h0 woven with du b0h1 [qm pool]
                et[(1, 0)] = etp.tile([P, JT * N], MM_DT, tag="et", name="et10")
                weave(sc_items(1, 0, et[(1, 0)]), du_chunks(0, 1, et[(0, 1)], "qm"))
                # P5: scores b1h1 woven with proj b0 + du b1h0 [ud] + du b1h1 half0 [qm]
                et[(1, 1)] = etp.tile([P, JT * N], MM_DT, tag="et", name="et11")
                du11 = du_chunks(1, 1, et[(1, 1)], "qm")
                weave(
                    sc_items(1, 1, et[(1, 1)]),
                    proj_chunks(0) + du_chunks(1, 0, et[(1, 0)], "ud") + du11[:6],
                )
                # P6: drain du b1h1 [qm]; proj b1 on the freed ud banks, with the
                # first two tiles' h0+identity matmuls filling the ao(1,1) wait.
                p1 = proj_chunks(1, pool="ud", split=2)
                for c in du11[6:]:
                    c()
                for c in p1:
                    c()
            else:
                for c in du_chunks(0, 1, et[(0, 1)], "qm"):
                    c()
                for c in proj_chunks(0):
                    c()

    nc.compile()
    return nc


def build_const_blob(gn_w, gn_b, wq, wk, wv, wp):
    """Returns (cbw bf16 [P, CB_W], cbg f32 [P, CB_G])."""
    import ml_dtypes

    cbw = np.zeros((P, CB_W), np.float32)
    for i, wmat in enumerate((wq, wk, wv, wp)):
        wT = np.asarray(wmat, np.float32).T  # (c_in, c_out)
        for kt in range(CT):
            cbw[:, OFF_W + i * CT * C + kt * C : OFF_W + i * CT * C + (kt + 1) * C] = (
                wT[kt * P : (kt + 1) * P, :]
            )
    cbw[:, OFF_ONES : OFF_ONES + P] = 1.0
    cbw[:, OFF_ID : OFF_ID + P] = np.eye(P, dtype=np.float32)
    cbg = np.zeros((P, CB_G), np.float32)
    cbg[:, OFF_GNWB + 0 : OFF_GNWB + 4 : 2] = np.asarray(gn_w, np.float32).reshape(CT, P).T
    cbg[:, OFF_GNWB + 1 : OFF_GNWB + 4 : 2] = np.asarray(gn_b, np.float32).reshape(CT, P).T
    for ct in range(CT):
        for p in range(P):
            g = (ct * P + p) // GSIZE
            cbg[p, OFF_GMASK + ct * G + g] = 1.0 / NG
            cbg[g, OFF_GMT + ct * P + p] = 1.0
    cbg[0:G, OFF_EPS] = EPS
    return cbw.astype(ml_dtypes.bfloat16), cbg


_NC_CACHE = {}


def kernel(x, gn_w, gn_b, wq, wk, wv, wp):
    import ml_dtypes

    x = np.ascontiguousarray(np.asarray(x, dtype=np.float32))
    b, c, h, w = x.shape
    xrb = x.reshape(b, c, h * w).astype(ml_dtypes.bfloat16)
    cbw, cbg = build_const_blob(gn_w, gn_b, wq, wk, wv, wp)

    if "nc" not in _NC_CACHE:
        _NC_CACHE["nc"] = build_bass()
    nc = _NC_CACHE["nc"]

    in_maps = [
        dict(
            xb=np.ascontiguousarray(xrb[i * BPC : (i + 1) * BPC]),
            cbw=cbw,
            cbg=cbg,
        )
        for i in range(N_CORES)
    ]
    res = run_bass_kernel_spmd(nc, in_maps, list(range(N_CORES)))
    out = np.concatenate([res.results[i]["out"] for i in range(N_CORES)], axis=0)
    return out.reshape(b, c, h, w).astype(np.float32)


if __name__ == "__main__":
    rng = np.random.default_rng(0)
    ins = {
        "x": rng.standard_normal((B, C, H, W), dtype=np.float32),
        "gn_w": np.ones((C,), np.float32),
        "gn_b": np.zeros((C,), np.float32),
        "wq": rng.standard_normal((C, C), dtype=np.float32) * C**-0.5,
        "wk": rng.standard_normal((C, C), dtype=np.float32) * C**-0.5,
        "wv": rng.standard_normal((C, C), dtype=np.float32) * C**-0.5,
        "wp": rng.standard_normal((C, C), dtype=np.float32) * C**-0.5,
    }
    out = kernel(**ins)
    print(out.shape, out.dtype)


# revision 31
# speedup vs baseline: 1.3742x; 1.1081x over previous
"""Trainium2 Bass kernel for nn_AttentionBlock (GroupNorm + 2-head attention + proj + residual).

Full inputs: x (16, 256, 32, 32) f32, gn_w/gn_b (256,), wq/wk/wv/wp (256, 256).
Sharding: pure data-parallel over batch - 16 / 8 cores = 2 batch elements per core.
No collectives; outputs concatenated on host.

Per-core dataflow (per batch element, channels-on-partitions):
  xb (256, 1024) bf16 -> GroupNorm stats on DVE (reduce_sum + fused
  tensor_tensor_reduce for sum-of-squares), group combine via tiny PE matmuls,
  xn = xb*scale+bias on DVE (per-partition affine). q,k = Wq/Wk @ xn; vT tiles =
  xn_mt^T @ Wv. Attention per head: st_jt (j=128, i=1024) = k_jt^T q;
  et = exp(scale*st) on ACT; U (hd, i) and D (softmax denom, replicated)
  accumulate over jt in PSUM as (128, 512) half-tiles; ao = U * recip(D) on DVE.
  proj: out_psum = Wp_h0^T ao0 + Wp_h1^T ao1 + I^T xb (residual folded in as an
  identity matmul on the bf16 x), copied to SBUF and DMA'd out.

Scheduling (emission order = per-engine queue order):
  - input DMAs: xb tiles lead each queue (cbg/cbw behind them) so GN stats
    start ASAP; a dummy Sqrt preloads the ACT sqrt table before it's needed.
  - 9 cold warmup MMs trip the HAM clock gate, 6 bridge MMs abut the first QKV.
  - scores of one head weave instruction-by-instruction with U/D matmuls of the
    previous head / QKV of the next batch, so PE never waits on ACT's exp.
  - per-head U/D PSUM alternates between the 'ud' pool (head 0) and the 'qm'
    pool (head 1) so consecutive heads' U/D accumulations overlap.
PSUM budget (8 banks): st (128,1024)x2bufs = 4, u+d (128,512) = 2, qm x2 = 2.
"""

import numpy as np

import concourse.bass as bass
import concourse.tile as tile
from concourse import bacc, mybir
from concourse.bass_utils import run_bass_kernel_spmd

N_CORES = 8
B = 16
BPC = B // N_CORES  # batch elements per core
C = 256
H = W = 32
N = H * W  # 1024 spatial positions
HEADS = 2
HD = C // HEADS  # 128 head dim
G = 4  # groupnorm groups
GSIZE = C // G  # 64 channels per group
EPS = 1e-5
ATT_SCALE = float((C * HEADS) ** -0.5)
P = 128  # partitions
CT = C // P  # channel tiles (2)
FT = 512  # u/d half-tile free dim
JT = N // P  # j tiles (8)
NG = GSIZE * N  # elements per (batch, group)

# const blob column offsets; [0, CB_W) bf16 weight blob, [0, CB_G) fp32 GN blob.
OFF_W = 0  # 4 weights (q,k,v,p), each CT*C = 512 cols
OFF_ONES = 2048  # 128 cols of 1.0
OFF_ID = 2176  # 128x128 identity
CB_W = 2304
OFF_GNWB = 0  # per ct: 2 cols (gn_w, gn_b)
OFF_GMASK = 4  # per ct: G cols (group mask / NG)
OFF_GMT = 12  # per ct: 128 cols (mask^T, values in rows 0..G-1)
OFF_EPS = 268  # one col: EPS in rows 0..G-1
CB_G = 269

f32 = mybir.dt.float32
bf16 = mybir.dt.bfloat16
fp8 = mybir.dt.float8e4
MM_DT = bf16
USE_FP8_DU = True  # exp output + U/D matmuls in fp8 with DoubleRow (2 j-tiles/MM)
ET_DT = fp8 if USE_FP8_DU else bf16
N_WARM1 = 12  # cold warmup MMs before the GN matmuls
N_WARM2 = 8  # bridge MMs between GN matmuls and first QKV matmul
AF = mybir.ActivationFunctionType
ALU = mybir.AluOpType
AX = mybir.AxisListType
USE_TTR = False  # tensor_tensor_reduce crashes TRN2 hw (NRT exec-unit error); use ACT Square


def build_bass(bpc=BPC):
    nc = bacc.Bacc("TRN2", target_bir_lowering=False, debug=False)

    xb_d = nc.dram_tensor("xb", [bpc, C, N], bf16, kind="ExternalInput").ap()
    cbw_d = nc.dram_tensor("cbw", [P, CB_W], MM_DT, kind="ExternalInput").ap()
    cbg_d = nc.dram_tensor("cbg", [P, CB_G], f32, kind="ExternalInput").ap()
    out_d = nc.dram_tensor("out", [bpc, C, N], f32, kind="ExternalOutput").ap()

    with tile.TileContext(nc) as tc:
        with (
            tc.tile_pool(name="consts", bufs=1) as consts,
            tc.tile_pool(name="xp", bufs=1) as xp,
            tc.tile_pool(name="xnp", bufs=1) as xnp,
            tc.tile_pool(name="qkp", bufs=1) as qkp,
            tc.tile_pool(name="vp", bufs=1) as vp,
            tc.tile_pool(name="etp", bufs=2) as etp,
            tc.tile_pool(name="aop", bufs=2) as aop,
            tc.tile_pool(name="smp", bufs=2) as smp,
            tc.tile_pool(name="pst", bufs=2, space="PSUM") as pst,
            tc.tile_pool(name="pud", bufs=1, space="PSUM") as pud,
            tc.tile_pool(name="pqm", bufs=2, space="PSUM") as pqm,
        ):
            # ---- SBUF constants + input DMAs.
            wt = consts.tile([P, FT], bf16, tag="warm")
            nc.gpsimd.memset(wt[:], 0.0)

            xbt = []
            for b in range(bpc):
                t = xp.tile([P, CT * N], bf16, tag=f"xb{b}", name=f"xb{b}")
                xbt.append(t)
            # xb tiles lead each queue; big/late consts behind them.
            nc.sync.dma_start(xbt[0][:, 0:N], xb_d[0, 0:P, :])
            cbw = consts.tile([P, CB_W], MM_DT, tag="cbw")
            nc.scalar.dma_start(xbt[0][:, N : 2 * N], xb_d[0, P : 2 * P, :])
            nc.scalar.dma_start(cbw[:], cbw_d[:])
            cbg = consts.tile([P, CB_G], f32, tag="cbg")
            nc.sync.dma_start(cbg[:], cbg_d[:])
            if bpc > 1:
                nc.scalar.dma_start(xbt[1][:, 0:N], xb_d[1, 0:P, :])
                nc.scalar.dma_start(xbt[1][:, N : 2 * N], xb_d[1, P : 2 * P, :])

            # dummy Sqrt: preload the sqrt ACT table before the GN chain needs it
            dum = smp.tile([G, 1], f32, tag="dum")
            nc.scalar.activation(dum[:], wt[0:G, 0:1], AF.Sqrt)

            def w_ap(i, kt):  # (128, C) lhsT slice of weight i, k-tile kt
                base = OFF_W + i * (CT * C) + kt * C
                return cbw[:, base : base + C]

            ones_ap = cbw[:, OFF_ONES : OFF_ONES + P]
            id_ap = cbw[:, OFF_ID : OFF_ID + P]
            gw = [cbg[:, OFF_GNWB + ct * 2 : OFF_GNWB + (ct + 1) * 2] for ct in range(CT)]
            gm = [cbg[:, OFF_GMASK + ct * G : OFF_GMASK + (ct + 1) * G] for ct in range(CT)]
            gmt = [cbg[0:G, OFF_GMT + ct * P : OFF_GMT + (ct + 1) * P] for ct in range(CT)]
            eps_ap = cbg[0:G, OFF_EPS : OFF_EPS + 1]
            WQ, WK, WV, WP_ = 0, 1, 2, 3

            # ---- warmup MMs (cold): trip the HAM clock gate.
            wps1 = pst.tile([P, FT], f32, tag="st")
            for _ in range(N_WARM1):
                nc.tensor.matmul(wps1[:], wt[:, 0:P], wt[:], start=True, stop=True)

            # ---- GroupNorm --------------------------------------------------
            s12_all = {}

            def gn_stats(b, s1_eng="dve"):
                """s1_eng='act' keeps batch-1 stats entirely off the DVE so the
                compiler cannot hoist them ahead of batch-0's GN chain ops."""
                s12s = []
                for ct in range(CT):
                    xsl = xbt[b][:, ct * N : (ct + 1) * N]
                    s12 = smp.tile([P, 2], f32, tag=f"s12_{ct}")
                    sq = smp.tile([P, N], f32, tag="sq")
                    if s1_eng == "act":
                        cp = smp.tile([P, N], bf16, tag="cp")
                        nc.scalar.activation(cp[:], xsl, AF.Copy, accum_out=s12[:, 0:1])
                    else:
                        nc.vector.reduce_sum(s12[:, 0:1], xsl, AX.X)
                    nc.scalar.activation(sq[:], xsl, AF.Square, accum_out=s12[:, 1:2])
                    s12s.append(s12)
                s12_all[b] = s12s

            def gn_mm1(b):
                gstats = pqm.tile([G, 2], f32, tag="qm")
                for ct in range(CT):
                    nc.tensor.matmul(
                        gstats[:], gm[ct], s12_all[b][ct][:],
                        start=(ct == 0), stop=(ct == CT - 1),
                    )
                return gstats

            def gn_chain_pre(b, gstats, defer_recip=False):
                mrs = smp.tile([G, 2], f32, tag="mrs")  # col0 = rstd, col1 = mean
                nc.vector.tensor_copy(mrs[:, 1:2], gstats[:, 0:1])
                negvar = smp.tile([G, 1], f32, tag="negvar")
                nc.vector.scalar_tensor_tensor(
                    negvar[:], mrs[:, 1:2], mrs[:, 1:2], gstats[:, 1:2],
                    ALU.mult, ALU.subtract,
                )
                std = smp.tile([G, 1], f32, tag="std")
                nc.scalar.activation(std[:], negvar[:], AF.Sqrt, bias=eps_ap, scale=-1.0)
                if defer_recip:
                    return (mrs, std)
                nc.vector.reciprocal(mrs[:, 0:1], std[:])
                return mrs

            def gn_post(b, mrs, xn_t):
                """bc matmuls + scale/bias + xn (DVE)."""
                for ct in range(CT):
                    bc = pqm.tile([P, 2], f32, tag="qm")
                    nc.tensor.matmul(bc[:], gmt[ct], mrs[:], start=True, stop=True)
                    scale = smp.tile([P, 1], f32, tag=f"scale{ct}")
                    nc.vector.tensor_tensor(scale[:], bc[:, 0:1], gw[ct][:, 0:1], ALU.mult)
                    nbias = smp.tile([P, 1], f32, tag=f"nbias{ct}")
                    nc.vector.tensor_tensor(nbias[:], bc[:, 1:2], scale[:], ALU.mult)
                    nc.vector.tensor_tensor(nbias[:], gw[ct][:, 1:2], nbias[:], ALU.subtract)
                    for nt in range(2):
                        sl = slice(ct * N + nt * FT, ct * N + (nt + 1) * FT)
                        nc.vector.tensor_scalar(
                            xn_t[:, sl], xbt[b][:, sl], scale[:], nbias[:],
                            ALU.mult, ALU.add,
                        )

            xn_all = {}

            # ---- QKV pieces -------------------------------------------------
            q_t, k_t, vT = {}, {}, {}

            def alloc_qk(b):
                q_t[b] = [qkp.tile([P, N], MM_DT, tag=f"q{b}{ot}", name=f"q{b}{ot}") for ot in range(CT)]
                k_t[b] = [qkp.tile([P, N], MM_DT, tag=f"k{b}{ot}", name=f"k{b}{ot}") for ot in range(CT)]

            def qk_chunk(b, wi, ot, cast_eng):
                """one (128,1024) psum + cast chunk for q or k, out tile ot."""
                dst = (q_t if wi == WQ else k_t)[b][ot]
                def c():
                    ps = pst.tile([P, N], f32, tag="st")
                    for nt in range(2):
                        sl = slice(nt * FT, (nt + 1) * FT)
                        for kt in range(CT):
                            nc.tensor.matmul(
                                ps[:, sl],
                                w_ap(wi, kt)[:, ot * P : (ot + 1) * P],
                                xn_all[b][:, kt * N + nt * FT : kt * N + (nt + 1) * FT],
                                start=(kt == 0), stop=(kt == CT - 1),
                            )
                    if cast_eng == "act":
                        nc.scalar.copy(dst[:], ps[:])
                    else:
                        nc.vector.tensor_copy(dst[:], ps[:])
                return c

            ones8 = None
            if USE_FP8_DU:
                ones8 = consts.tile([P, 2 * P], fp8, tag="ones8")
                nc.vector.tensor_copy(ones8[:, 0:P], ones_ap)
                nc.vector.tensor_copy(ones8[:, P : 2 * P], ones_ap)

            def v_chunks(b):
                vT[b] = vp.tile([P, JT * C], ET_DT, tag=f"vt{b}", name=f"vt{b}")
                chunks = []
                for mt0 in range(0, JT, 2):
                    def c(mt0=mt0, b=b):
                        for mt in (mt0, mt0 + 1):
                            ps = pqm.tile([P, C], f32, tag="qm")
                            for kt in range(CT):
                                nc.tensor.matmul(
                                    ps[:],
                                    xn_all[b][:, kt * N + mt * P : kt * N + (mt + 1) * P],
                                    w_ap(WV, kt),
                                    start=(kt == 0), stop=(kt == CT - 1),
                                )
                            nc.vector.tensor_copy(vT[b][:, mt * C : (mt + 1) * C], ps[:])
                    chunks.append(c)
                return chunks

            # ---- attention pieces ------------------------------------------
            def sc_items(b, h, et_tile, act_extras=None):
                """8 items; act_extras[jt] = list of ACT closures to emit after
                exp jt (used to slot sqrt/k-casts into the exp stream)."""
                items = []
                for jt in range(JT):
                    def s(jt=jt, b=b, h=h, et_tile=et_tile):
                        st = pst.tile([P, N], f32, tag="st")
                        for nt in range(2):
                            sl = slice(nt * FT, (nt + 1) * FT)
                            nc.tensor.matmul(
                                st[:, sl],
                                k_t[b][h][:, jt * P : (jt + 1) * P],
                                q_t[b][h][:, sl],
                                start=True, stop=True,
                            )
                        nc.scalar.activation(
                            et_tile[:, jt * N : (jt + 1) * N], st[:],
                            AF.Exp, scale=ATT_SCALE,
                        )
                        if act_extras and jt in act_extras:
                            for fn in act_extras[jt]:
                                fn()
                    items.append(s)
                return items

            ao_t = {}

            def alloc_ud(ud, half, pool_tag):
                if pool_tag == "ud":
                    ud[half] = (
                        pud.tile([P, FT], f32, tag="u", name="u"),
                        pud.tile([P, FT], f32, tag="d", name="d"),
                    )
                else:
                    ud[half] = (
                        pqm.tile([P, FT], f32, tag="qm", name="uq"),
                        pqm.tile([P, FT], f32, tag="qm", name="dq"),
                    )

            def du_chunks(b, h, et_tile, pool_tag):
                """U/D accumulation chunks per half + 1 ao chunk (DVE) per half.
                bf16: one (d,u) MM pair per jt. fp8: DoubleRow consumes a PAIR of
                jt tiles per MM (contraction 256), halving PE streaming time."""
                if (b, h) not in ao_t:
                    ao_t[(b, h)] = aop.tile([P, N], MM_DT, tag=f"ao{h}", name=f"ao{b}{h}")
                ao = ao_t[(b, h)]
                ud = {}
                chunks = []
                if USE_FP8_DU:
                    et3 = et_tile.rearrange("p (a b n) -> p a b n", a=JT // 2, b=2)
                    vt3 = vT[b].rearrange("p (a b c) -> p a b c", a=JT // 2, b=2)
                    on3 = ones8.rearrange("p (b m) -> p b m", b=2)
                    DR = mybir.MatmulPerfMode.DoubleRow
                for half in range(2):
                    if USE_FP8_DU:
                        for pr in range(JT // 2):
                            def c(half=half, pr=pr, b=b, h=h):
                                if pr == 0:
                                    alloc_ud(ud, half, pool_tag)
                                u_, d_ = ud[half]
                                rhs = et3[:, pr, :, half * FT : (half + 1) * FT]
                                nc.tensor.matmul(
                                    d_[:], on3[:], rhs,
                                    start=(pr == 0), stop=(pr == JT // 2 - 1),
                                    perf_mode=DR,
                                )
                                nc.tensor.matmul(
                                    u_[:], vt3[:, pr, :, h * HD : (h + 1) * HD], rhs,
                                    start=(pr == 0), stop=(pr == JT // 2 - 1),
                                    perf_mode=DR,
                                )
                            chunks.append(c)
                    else:
                        for jt in range(JT):
                            def c(half=half, jt=jt, b=b, h=h, et_tile=et_tile):
                                if jt == 0:
                                    alloc_ud(ud, half, pool_tag)
                                u_, d_ = ud[half]
                                sl = et_tile[:, jt * N + half * FT : jt * N + half * FT + FT]
                                nc.tensor.matmul(
                                    d_[:], ones_ap, sl, start=(jt == 0), stop=(jt == JT - 1)
                                )
                                nc.tensor.matmul(
                                    u_[:],
                                    vT[b][:, jt * C + h * HD : jt * C + h * HD + HD],
                                    sl,
                                    start=(jt == 0), stop=(jt == JT - 1),
                                )
                            chunks.append(c)
                    def ao_c(half=half):
                        u_, d_ = ud[half]
                        r = smp.tile([P, FT], f32, tag="r")
                        nc.vector.reciprocal_approx_fast(out=r[:], in_=d_[:])
                        nc.vector.tensor_tensor(
                            ao[:, half * FT : (half + 1) * FT], u_[:], r[:], ALU.mult
                        )
                    chunks.append(ao_c)
                return chunks

            def proj_chunks(b, pool="qm", split=0):
                """split>0: for the first `split` (ot,nt) tiles, emit the
                h0+identity matmuls as early chunks and defer the h1 matmul +
                copy + DMA, so the PE can fill the last softmax-normalize wait."""
                engs = [nc.sync, nc.gpsimd] if b == 0 else [nc.sync, nc.scalar]
                tags = ["qm", "qm"] if pool == "qm" else ["u", "d"]
                heads_t, tails_t = [], []
                for idx, (ot, nt) in enumerate([(o, n) for o in range(CT) for n in range(2)]):
                    state = {}
                    def c_head(ot=ot, nt=nt, b=b, idx=idx, state=state):
                        if pool == "qm":
                            pj = pqm.tile([P, FT], f32, tag="qm", name="pj")
                        else:
                            pj = pud.tile([P, FT], f32, tag=tags[idx % 2], name="pj")
                        state["pj"] = pj
                        nc.tensor.matmul(
                            pj[:],
                            w_ap(WP_, 0)[:, ot * P : (ot + 1) * P],
                            ao_t[(b, 0)][:, nt * FT : (nt + 1) * FT],
                            start=True, stop=False,
                        )
                        nc.tensor.matmul(
                            pj[:], id_ap,
                            xbt[b][:, ot * N + nt * FT : ot * N + (nt + 1) * FT],
                            start=False, stop=False,
                        )
                    def c_tail(ot=ot, nt=nt, b=b, idx=idx, state=state):
                        pj = state["pj"]
                        sl = slice(nt * FT, (nt + 1) * FT)
                        nc.tensor.matmul(
                            pj[:],
                            w_ap(WP_, 1)[:, ot * P : (ot + 1) * P],
                            ao_t[(b, 1)][:, sl],
                            start=False, stop=True,
                        )
                        ot_sb = smp.tile([P, FT], f32, tag="osb", name="osb")
                        nc.vector.tensor_copy(ot_sb[:], pj[:])
                        engs[idx % 2].dma_start(
                            out_d[b, ot * P : (ot + 1) * P, sl], ot_sb[:]
                        )
                    if idx < split:
                        heads_t.append(c_head)
                        tails_t.append(c_tail)
                    else:
                        def c_full(ch=c_head, ct_=c_tail):
                            ch()
                            ct_()
                        tails_t.append(c_full)
                return heads_t + tails_t

            def weave(score_it, fillers):
                fi = 0
                ns = max(1, len(score_it))
                for i, s in enumerate(score_it):
                    s()
                    target = (i + 1) * len(fillers) // ns
                    while fi < target:
                        fillers[fi]()
                        fi += 1
                while fi < len(fillers):
                    fillers[fi]()
                    fi += 1

            # ---- global schedule -------------------------------------------
            # GN b0 (DVE stats overlap warmup MMs)
            gn_stats(0)
            gst0 = gn_mm1(0)
            mrs0 = gn_chain_pre(0, gst0)
            xn_all[0] = xnp.tile([P, CT * N], bf16, tag="xn0", name="xn0")
            gn_post(0, mrs0, xn_all[0])

            wps2 = pst.tile([P, FT], f32, tag="st")
            for _ in range(N_WARM2):
                nc.tensor.matmul(wps2[:], wt[:, 0:P], wt[:], start=True, stop=True)

            # q/k of b0 head 0 first (unblocks scores b0h0), casts on DVE
            alloc_qk(0)
            qk_chunk(0, WQ, 0, "dve")()
            qk_chunk(0, WK, 0, "dve")()

            # b1 GN stats on ACT (nothing on DVE to hoist); v0 matmuls keep PE busy
            v0 = v_chunks(0)
            if bpc > 1:
                gn_stats(1, s1_eng="act")
            for c in v0[:2]:
                c()

            mrs1_box = {}
            if bpc > 1:
                # gstats1 matmul + sqrt1 BEFORE the first exp: keeps the ACT
                # sqrt/square table resident until all sqrt work is done, so the
                # exp table set loads exactly once.
                gst1 = gn_mm1(1)
                mrs1_box["mrs"], mrs1_box["std"] = gn_chain_pre(1, gst1, defer_recip=True)

            def gn1_post_chunk():
                if bpc > 1:
                    mrs1 = mrs1_box["mrs"]
                    nc.vector.reciprocal(mrs1[:, 0:1], mrs1_box["std"][:])
                    xn_all[1] = xnp.tile([P, CT * N], bf16, tag="xn1", name="xn1")
                    gn_post(1, mrs1, xn_all[1])

            # P2: scores b0h0 woven with v0 rest, q/k ot1, gn1 post, qkv b1
            et = {}
            et[(0, 0)] = etp.tile([P, JT * N], ET_DT, tag="et", name="et00")
            fill = []
            fill += v0[2:]
            fill.append(qk_chunk(0, WQ, 1, "dve"))
            fill.append(qk_chunk(0, WK, 1, "dve"))
            fill.append(gn1_post_chunk)
            if bpc > 1:
                alloc_qk(1)
                fill.append(qk_chunk(1, WQ, 0, "dve"))
                fill.append(qk_chunk(1, WK, 0, "dve"))
                fill.append(qk_chunk(1, WQ, 1, "dve"))
                fill.append(qk_chunk(1, WK, 1, "dve"))
                fill += v_chunks(1)
            weave(sc_items(0, 0, et[(0, 0)]), fill)

            # P3: scores b0h1 woven with du b0h0 [ud pool]
            et[(0, 1)] = etp.tile([P, JT * N], ET_DT, tag="et", name="et01")
            weave(sc_items(0, 1, et[(0, 1)]), du_chunks(0, 0, et[(0, 0)], "ud"))
            if bpc > 1:
                # P4: scores b1h0 woven with du b0h1 [qm pool]
                et[(1, 0)] = etp.tile([P, JT * N], ET_DT, tag="et", name="et10")
                weave(sc_items(1, 0, et[(1, 0)]), du_chunks(0, 1, et[(0, 1)], "qm"))
                # P5: scores b1h1 woven with proj b0 + du b1h0 [ud] + du b1h1 half0 [qm]
                et[(1, 1)] = etp.tile([P, JT * N], ET_DT, tag="et", name="et11")
                du11 = du_chunks(1, 1, et[(1, 1)], "qm")
                keep5 = (JT // 2 if USE_FP8_DU else JT) - 2
                weave(
                    sc_items(1, 1, et[(1, 1)]),
                    proj_chunks(0) + du_chunks(1, 0, et[(1, 0)], "ud") + du11[:keep5],
                )
                # P6: drain du b1h1 [qm]; proj b1 on the freed ud banks, with the
                # first two tiles' h0+identity matmuls filling the ao(1,1) wait.
                p1 = proj_chunks(1, pool="ud", split=2)
                for c in du11[keep5:]:
                    c()
                for c in p1:
                    c()
            else:
                for c in du_chunks(0, 1, et[(0, 1)], "qm"):
                    c()
                for c in proj_chunks(0):
                    c()

    nc.compile()
    return nc


def build_const_blob(gn_w, gn_b, wq, wk, wv, wp):
    """Returns (cbw bf16 [P, CB_W], cbg f32 [P, CB_G])."""
    import ml_dtypes

    cbw = np.zeros((P, CB_W), np.float32)
    for i, wmat in enumerate((wq, wk, wv, wp)):
        wT = np.asarray(wmat, np.float32).T  # (c_in, c_out)
        for kt in range(CT):
            cbw[:, OFF_W + i * CT * C + kt * C : OFF_W + i * CT * C + (kt + 1) * C] = (
                wT[kt * P : (kt + 1) * P, :]
            )
    cbw[:, OFF_ONES : OFF_ONES + P] = 1.0
    cbw[:, OFF_ID : OFF_ID + P] = np.eye(P, dtype=np.float32)
    cbg = np.zeros((P, CB_G), np.float32)
    cbg[:, OFF_GNWB + 0 : OFF_GNWB + 4 : 2] = np.asarray(gn_w, np.float32).reshape(CT, P).T
    cbg[:, OFF_GNWB + 1 : OFF_GNWB + 4 : 2] = np.asarray(gn_b, np.float32).reshape(CT, P).T
    for ct in range(CT):
        for p in range(P):
            g = (ct * P + p) // GSIZE
            cbg[p, OFF_GMASK + ct * G + g] = 1.0 / NG
            cbg[g, OFF_GMT + ct * P + p] = 1.0
    cbg[0:G, OFF_EPS] = EPS
    return cbw.astype(ml_dtypes.bfloat16), cbg


_NC_CACHE = {}


def kernel(x, gn_w, gn_b, wq, wk, wv, wp):
    import ml_dtypes

    x = np.ascontiguousarray(np.asarray(x, dtype=np.float32))
    b, c, h, w = x.shape
    xrb = x.reshape(b, c, h * w).astype(ml_dtypes.bfloat16)
    cbw, cbg = build_const_blob(gn_w, gn_b, wq, wk, wv, wp)

    if "nc" not in _NC_CACHE:
        _NC_CACHE["nc"] = build_bass()
    nc = _NC_CACHE["nc"]

    in_maps = [
        dict(
            xb=np.ascontiguousarray(xrb[i * BPC : (i + 1) * BPC]),
            cbw=cbw,
            cbg=cbg,
        )
        for i in range(N_CORES)
    ]
    res = run_bass_kernel_spmd(nc, in_maps, list(range(N_CORES)))
    out = np.concatenate([res.results[i]["out"] for i in range(N_CORES)], axis=0)
    return out.reshape(b, c, h, w).astype(np.float32)


if __name__ == "__main__":
    rng = np.random.default_rng(0)
    ins = {
        "x": rng.standard_normal((B, C, H, W), dtype=np.float32),
        "gn_w": np.ones((C,), np.float32),
        "gn_b": np.zeros((C,), np.float32),
        "wq": rng.standard_normal((C, C), dtype=np.float32) * C**-0.5,
        "wk": rng.standard_normal((C, C), dtype=np.float32) * C**-0.5,
        "wv": rng.standard_normal((C, C), dtype=np.float32) * C**-0.5,
        "wp": rng.standard_normal((C, C), dtype=np.float32) * C**-0.5,
    }
    out = kernel(**ins)
    print(out.shape, out.dtype)
